# revision 14
# baseline (speedup 1.0000x reference)
"""Trainium2 Bass kernel for nn_CrossAttentionBlock (B=2, S=2048, D=1024, H=16, HD=64).

Sharding: 8 cores = 2 batches x 4 head-quads (4 heads each, E=256 channels).
Each core computes q/k/v projections for its quad, RoPE, SDPA, and a partial
output projection [S, D]; host sums the 4 partials per batch and adds bo.

v2 pipeline (vs the phase-serial baseline):
  - The SDPA inner loop is software-pipelined: scores(ki+1) is emitted before
    ctx(ki) so the PE never head-of-line blocks on the ScalarE exp.  ScalarE
    does nothing but exp (the hard ~147us floor); all PSUM->SBUF copies are on
    DVE, and the projections / output projection are interleaved into the
    exp-bound window as PE filler work.
  - Pair-major ki loop: pair 0's exp tiles are kept in a persistent SBUF ring
    so softmax denominators run as 4-way col-tiled concurrent matmuls in pair
    1's loop.  PSUM: scores ping-pong 4 banks + ctx 2 + den 1 + aux 1 = 8.
  - Inputs are host-packed so each s-chunk is a single DMA; the DMA order is
    prioritized so scores/exp start ~17us in instead of ~40us.
  - RoPE uses a host-permuted sin table: u = q*sinP read straight from PSUM,
    partition-swap of u via SBUF-SBUF DMA, one DVE add.  No ScalarE copies.
"""
import collections
import os
import sys

sys.path.insert(0, "/opt/trn_rl_repo")

import numpy as np
import ml_dtypes

BF16 = ml_dtypes.bfloat16

B, S, D, H = 2, 2048, 1024, 16
HD = D // H          # 64
DIM = HD // 2        # 32
QUADS = 4
E = D // QUADS       # 256 channels per core
ROPE_BASE = 10000.0
N_CORES = 8

KT = D // 128        # 8 d k-tiles
ST = S // 128        # 16 s-tiles
QC = S // 512        # 4 s-chunks
CW = KT * 512        # 4096 packed x cols per s-chunk


def _pack_x(xT):
    """[D(+1), S] d-major -> [128, QC*CW] s-chunk-major packed layout.

    packed[p, sc*CW + k*512 + s] = xT[k*128 + p, sc*512 + s]
    (bias row, if present, is returned separately)
    """
    xmain = xT[:D].reshape(KT, 128, QC, 512)
    packed = xmain.transpose(1, 2, 0, 3).reshape(128, QC * CW)
    return np.ascontiguousarray(packed)


def _pack_w(wT):
    """[D(+1), E] -> [128, KT*E]: packed[p, k*E + c] = wT[k*128 + p, c]."""
    return np.ascontiguousarray(
        wT[:D].reshape(KT, 128, E).transpose(1, 0, 2).reshape(128, KT * E))


def _host_prep(x_q, x_kv, wq, bq, wk, bk, wv, bv, wo, with_bias):
    perm = np.concatenate([np.arange(0, HD, 2), np.arange(1, HD, 2)])  # even|odd
    scale = 1.0 / np.sqrt(HD)

    freqs = np.exp(-np.arange(DIM, dtype=np.float64) * np.log(ROPE_BASE) / DIM)
    ang = np.arange(S, dtype=np.float64)[:, None] * freqs[None, :]     # [S, 32]
    cos = np.cos(ang).T                                                # [32, S]
    sin = np.sin(ang).T
    # rot-half: dst = q*cos64 + swap(q)*sin64, cos64=[cos;cos], sin64=[-sin;sin]
    # permuted-sin trick: ship sinP = swap_rows(sin64) = [sin;-sin]; then
    # u = q*sinP and swap(u) = swap(q)*sin64 exactly.
    cos64 = np.concatenate([cos, cos], axis=0)                         # [64, S]
    sinp64 = np.concatenate([sin, -sin], axis=0)
    cosT = np.concatenate([cos64, cos64], axis=0).astype(BF16)         # [128, S]
    sinPT = np.concatenate([sinp64, sinp64], axis=0).astype(BF16)

    def proj_mat(w, b, permute, s):
        blocks, brows = [], []
        for h in range(4):
            rows = slice(h * HD, (h + 1) * HD)
            wb_ = w[rows, :]
            bb = b[rows]
            if permute:
                wb_ = wb_[perm, :]
                bb = bb[perm]
            blocks.append(wb_ * s)
            brows.append(bb * s)
        wstack = np.concatenate(blocks, axis=0)          # [256, 1024]
        bstack = np.concatenate(brows, axis=0)           # [256]
        return wstack.T, bstack                          # [1024, 256], [256]

    ones_row = np.ones((1, S), dtype=BF16)
    in_maps = []
    for c in range(N_CORES):
        b_ = c // QUADS
        g = c % QUADS
        hs = slice(g * E, (g + 1) * E)
        wqT_, bq_ = proj_mat(wq[hs, :], bq[hs], True, scale)
        wkT_, bk_ = proj_mat(wk[hs, :], bk[hs], True, 1.0)
        wvT_, bv_ = proj_mat(wv[hs, :], bv[hs], False, 1.0)
        m = {
            "xqT": _pack_x(x_q[b_].T.astype(BF16)),
            "xkvT": _pack_x(x_kv[b_].T.astype(BF16)),
            "wqT": _pack_w(wqT_.astype(BF16)),
            "wkT": _pack_w(wkT_.astype(BF16)),
            "wvT": _pack_w(wvT_.astype(BF16)),
            "woT": np.ascontiguousarray(
                wo[:, hs].T.reshape(2, 128, D).transpose(1, 0, 2).reshape(128, 2 * D)
            ).astype(BF16),
            "cosT": np.ascontiguousarray(cosT),
            "sinPT": np.ascontiguousarray(sinPT),
            "ones_col": np.ones((128, 1), dtype=BF16),
        }
        if with_bias:
            m["xqb"] = ones_row.copy()
            m["xkvb"] = ones_row.copy()
            m["wqb"] = bq_[None, :].astype(BF16)
            m["wkb"] = bk_[None, :].astype(BF16)
            m["wvb"] = bv_[None, :].astype(BF16)
        in_maps.append(m)
    return in_maps


# ---------------------------------------------------------------------------
_PROGRAM_CACHE = {}


def _fixed_tile_context(tile_mod, bass_rust_mod, vector_clock_mod):
    """TileContext whose tail drain splits multi-sem waits into single-wait
    NOPs (this walrus rejects >1 sync-wait on one instruction)."""
    SyncInfo = bass_rust_mod.SyncInfo
    ScopedClock = vector_clock_mod.ScopedClock

    class TC(tile_mod.TileContext):
        def _drain_and_barrier(self, tick_clock, wait_clock):
            harvest = self.nc.sync.nop(nofuse=True)
            wait_clock.add_sem_waits(
                harvest.ins, ScopedClock({None: tick_clock.global_clock}))
            si = harvest.ins.sync_info
            waits = list(si.on_wait) if si is not None else []
            if len(waits) > 1:
                harvest.ins.sync_info = SyncInfo(
                    on_wait=[waits[0]], on_update=list(si.on_update))
                for w in waits[1:]:
                    nop = self.nc.sync.nop(nofuse=True)
                    nop.ins.sync_info = SyncInfo(on_wait=[w], on_update=[])
            self.nc.sync.drain()
            self.nc.all_engine_barrier()
            assert self.sems is not None
            popped = self.nc._tile_sem_poison_stack.pop()
            assert popped is self._sem_poison
            self.nc.clear_and_free_semaphores(list(self.sems.allocated().values()))
            self.nc.all_engine_barrier()

    return TC


def _split_multiwait_instructions(nc, mybir, SyncInfo):
    """This walrus build rejects >1 sync-wait per instruction; hoist extra
    waits onto single-wait NOPs inserted just before, on the same engine."""
    ctr = 0
    for blk in nc.m.functions[0].blocks:
        insts = blk.instructions
        i = 0
        while i < len(insts):
            inst = insts[i]
            si = inst.sync_info
            if si is not None and len(si.on_wait) > 1:
                waits = list(si.on_wait)
                inst.sync_info = SyncInfo(on_wait=[waits[-1]],
                                          on_update=list(si.on_update))
                nops = []
                for w in waits[:-1]:
                    nop = mybir.InstNoOp(name=f"waitsplit_{ctr}", ins=[], outs=[])
                    ctr += 1
                    nop.engine = inst.engine
                    nop.sync_info = SyncInfo(on_wait=[w], on_update=[])
                    nops.append(nop)
                insts[i:i] = nops
                i += len(nops)
            i += 1
    return ctr


def build_program(with_bias=False):
    import concourse.bass as bass
    import concourse.mybir as mybir
    import concourse.tile as tile
    import bass_rust
    from concourse import vector_clock

    f32 = mybir.dt.float32
    bf16 = mybir.dt.bfloat16
    Exp = mybir.ActivationFunctionType.Exp
    mult = mybir.AluOpType.mult
    add = mybir.AluOpType.add

    nc = bass.Bass("TRN2", target_bir_lowering=False, debug=False,
                   num_devices=N_CORES)

    xqT = nc.dram_tensor("xqT", [128, QC * CW], bf16, kind="ExternalInput").ap()
    xkvT = nc.dram_tensor("xkvT", [128, QC * CW], bf16, kind="ExternalInput").ap()
    wqT = nc.dram_tensor("wqT", [128, KT * E], bf16, kind="ExternalInput").ap()
    wkT = nc.dram_tensor("wkT", [128, KT * E], bf16, kind="ExternalInput").ap()
    wvT = nc.dram_tensor("wvT", [128, KT * E], bf16, kind="ExternalInput").ap()
    woT = nc.dram_tensor("woT", [128, 2 * D], bf16, kind="ExternalInput").ap()
    cosT = nc.dram_tensor("cosT", [128, S], bf16, kind="ExternalInput").ap()
    sinPT = nc.dram_tensor("sinPT", [128, S], bf16, kind="ExternalInput").ap()
    ones_col = nc.dram_tensor("ones_col", [128, 1], bf16, kind="ExternalInput").ap()
    if with_bias:
        xqb = nc.dram_tensor("xqb", [1, S], bf16, kind="ExternalInput").ap()
        xkvb = nc.dram_tensor("xkvb", [1, S], bf16, kind="ExternalInput").ap()
        wqb = nc.dram_tensor("wqb", [1, E], bf16, kind="ExternalInput").ap()
        wkb = nc.dram_tensor("wkb", [1, E], bf16, kind="ExternalInput").ap()
        wvb = nc.dram_tensor("wvb", [1, E], bf16, kind="ExternalInput").ap()
    out = nc.dram_tensor("out", [S, D], f32, kind="ExternalOutput").ap()

    TC = _fixed_tile_context(tile, bass_rust, vector_clock)

    with TC(nc) as tc:
        with tc.tile_pool(name="per", bufs=1) as per, \
             tc.tile_pool(name="rp", bufs=2) as rp, \
             tc.tile_pool(name="esb", bufs=3) as esp, \
             tc.tile_pool(name="osb", bufs=2) as osb, \
             tc.tile_pool(name="nrm", bufs=2) as nrm:

            # ---- persistent tiles ----
            xq_sb = per.tile([128, QC * CW], bf16, tag="xq", name="xq")
            xkv_sb = per.tile([128, QC * CW], bf16, tag="xkv", name="xkv")
            wq_sb = per.tile([128, KT * E], bf16, tag="wq", name="wq")
            wk_sb = per.tile([128, KT * E], bf16, tag="wk", name="wk")
            wv_sb = per.tile([128, KT * E], bf16, tag="wv", name="wv")
            wo_sb = per.tile([128, 2 * D], bf16, tag="wo", name="wo")
            cos_sb = per.tile([128, S], bf16, tag="cos", name="cos")
            sinp_sb = per.tile([128, S], bf16, tag="sinp", name="sinp")
            ones_sb = per.tile([128, 1], bf16, tag="ones", name="ones")
            qr_sb = [per.tile([128, S], bf16, tag=f"qr{p}", name=f"qr{p}")
                     for p in range(2)]
            kr_sb = [per.tile([128, S], bf16, tag=f"kr{p}", name=f"kr{p}")
                     for p in range(2)]
            v_sb = per.tile([128, ST * E], bf16, tag="v", name="v")
            ctxn_sb = [per.tile([128, S], bf16, tag=f"ctxn{p}", name=f"ctxn{p}")
                       for p in range(2)]
            # pair-0 exp tiles for one qh, kept so dens run in pair 1's loop
            e0_sb = per.tile([128, ST * 1024], bf16, tag="e0", name="e0")
            ew_sb = per.tile([1, 1], f32, tag="ew", name="ew")
            if with_bias:
                xqb_sb = per.tile([1, S], bf16, tag="xqb", name="xqb")
                xkvb_sb = per.tile([1, S], bf16, tag="xkvb", name="xkvb")
                wqb_sb = per.tile([1, E], bf16, tag="wqb", name="wqb")
                wkb_sb = per.tile([1, E], bf16, tag="wkb", name="wkb")
                wvb_sb = per.tile([1, E], bf16, tag="wvb", name="wvb")

            # ---- input DMAs, priority order (critical path first) ----
            nc.sync.dma_start(wk_sb[:, :], wkT[:, :])
            nc.sync.dma_start(xkv_sb[:, 0:CW], xkvT[:, 0:CW])
            nc.sync.dma_start(wq_sb[:, :], wqT[:, :])
            nc.sync.dma_start(cos_sb[:, :], cosT[:, :])
            nc.sync.dma_start(sinp_sb[:, :], sinPT[:, :])
            nc.sync.dma_start(xq_sb[:, 0:CW], xqT[:, 0:CW])
            nc.sync.dma_start(wv_sb[:, :], wvT[:, :])
            if with_bias:
                nc.sync.dma_start(xkvb_sb[:, :], xkvb[:, :])
                nc.sync.dma_start(xqb_sb[:, :], xqb[:, :])
                nc.sync.dma_start(wkb_sb[:, :], wkb[:, :])
                nc.sync.dma_start(wqb_sb[:, :], wqb[:, :])
                nc.sync.dma_start(wvb_sb[:, :], wvb[:, :])
            for sc in range(1, QC):
                nc.sync.dma_start(xkv_sb[:, sc * CW:(sc + 1) * CW],
                                  xkvT[:, sc * CW:(sc + 1) * CW])
            for sc in range(1, QC):
                nc.sync.dma_start(xq_sb[:, sc * CW:(sc + 1) * CW],
                                  xqT[:, sc * CW:(sc + 1) * CW])
            nc.sync.dma_start(wo_sb[:, :], woT[:, :])
            nc.sync.dma_start(ones_sb[:, :], ones_col[:, :])

            # preload the exp table set during the DMA window
            nc.scalar.activation(ew_sb[0:1, 0:1], cos_sb[0:1, 0:1], Exp)

            # ---- helpers (each returns a list of ~0.7us micro-closures) ----
            def rope_chunk(q_ps, dst, sc):
                # dst = q*cos + swap32(q)*sin64.  The swap happens inside the
                # sin-multiply: out quadrant d0 reads PSUM quadrant s0 (mixed
                # space, so unequal base partitions are legal), and
                # sinP[s0..] == sin64[d0..] by host construction.
                scs = slice(sc * 512, (sc + 1) * 512)
                usw = rp.tile([128, 512], bf16, tag="u", name="usw")
                a = rp.tile([128, 512], bf16, tag="a", name="a")
                for d0, s0 in ((0, 32), (32, 0), (64, 96), (96, 64)):
                    nc.vector.tensor_tensor(
                        usw[d0:d0 + 32, :], q_ps[s0:s0 + 32, :],
                        sinp_sb[s0:s0 + 32, scs], mult)
                nc.vector.tensor_tensor(a[:, :], q_ps[:, :], cos_sb[:, scs], mult)
                nc.vector.tensor_tensor(dst[:, scs], a[:, :], usw[:, :], add)

            def proj_chunk(pool, tag, which, p, sc):
                if which == "q":
                    w_, src, dst = wq_sb, xq_sb, qr_sb
                    wb_, srcb_ = (wqb_sb, xqb_sb) if with_bias else (None, None)
                else:
                    w_, src, dst = wk_sb, xkv_sb, kr_sb
                    wb_, srcb_ = (wkb_sb, xkvb_sb) if with_bias else (None, None)
                cell = {}

                def mm_part(k0, k1):
                    if k0 == 0:
                        cell["ps"] = pool.tile([128, 512], f32, tag=tag,
                                               name=f"{which}proj")
                    q_ps = cell["ps"]
                    for k in range(k0, k1):
                        for h2 in range(2):
                            co = k * E + p * 128 + h2 * 64
                            nc.tensor.matmul(
                                q_ps[h2 * 64:(h2 + 1) * 64, :],
                                lhsT=w_[:, co:co + 64],
                                rhs=src[:, sc * CW + k * 512:
                                        sc * CW + (k + 1) * 512],
                                tile_position=(0, h2 * 64),
                                start=(k == 0),
                                stop=(k == KT - 1) and not with_bias,
                                skip_group_check=True)
                    if k1 == KT and with_bias:
                        nc.tensor.matmul(
                            q_ps[:, :], lhsT=wb_[:, p * 128:(p + 1) * 128],
                            rhs=srcb_[:, sc * 512:(sc + 1) * 512],
                            start=False, stop=True, skip_group_check=True)

                return [lambda k0=k0: mm_part(k0, k0 + 2) for k0 in (0, 2, 4, 6)] \
                    + [lambda: rope_chunk(cell["ps"], dst[p], sc)]

            def vproj_st(pool, tag, st):
                sc, si = st // 4, st % 4
                cell = {}

                def mm_part(k0, k1):
                    if k0 == 0:
                        cell["ps"] = pool.tile([128, E], f32, tag=tag,
                                               name="vproj")
                    v_ps = cell["ps"]
                    for k in range(k0, k1):
                        for h2 in range(2):
                            co = sc * CW + k * 512 + si * 128 + h2 * 64
                            nc.tensor.matmul(
                                v_ps[h2 * 64:(h2 + 1) * 64, :],
                                lhsT=xkv_sb[:, co:co + 64],
                                rhs=wv_sb[:, k * E:(k + 1) * E],
                                tile_position=(0, h2 * 64),
                                start=(k == 0),
                                stop=(k == KT - 1) and not with_bias,
                                skip_group_check=True)
                    if k1 == KT:
                        if with_bias:
                            nc.tensor.matmul(
                                v_ps[:, :],
                                lhsT=xkvb_sb[:, st * 128:(st + 1) * 128],
                                rhs=wvb_sb[:, :],
                                start=False, stop=True, skip_group_check=True)
                        nc.vector.tensor_copy(
                            v_sb[:, st * E:(st + 1) * E], v_ps[:, :])

                return [lambda: mm_part(0, 4), lambda: mm_part(4, KT)]

            def outproj_chunk(pool, tag, st, dch):
                cell = {}

                def mm_part(p):
                    if p == 0:
                        cell["ps"] = pool.tile([128, 512], f32, tag=tag,
                                               name="oproj")
                    o_ps = cell["ps"]
                    for h2 in range(2):
                        nc.tensor.matmul(
                            o_ps[h2 * 64:(h2 + 1) * 64, :],
                            lhsT=ctxn_sb[p][:, st * 128 + h2 * 64:
                                            st * 128 + h2 * 64 + 64],
                            rhs=wo_sb[:, p * D + dch * 512:
                                      p * D + (dch + 1) * 512],
                            tile_position=(0, h2 * 64),
                            start=(p == 0), stop=(p == 1),
                            skip_group_check=True)
                    if p == 1:
                        o_t = osb.tile([128, 512], f32, tag="o", name="o")
                        nc.vector.tensor_copy(o_t[:, :], o_ps[:, :])
                        nc.sync.dma_start(
                            out[st * 128:(st + 1) * 128,
                                dch * 512:(dch + 1) * 512], o_t[:, :])

                return [lambda: mm_part(0), lambda: mm_part(1)]

            def run_all(parts):
                for f in parts:
                    f()

            # ---- lead-in (own PSUM pool, released before SDPA) ----
            with tc.tile_pool(name="pps", bufs=2, space="PSUM") as pps:
                wps = pps.tile([128, 512], f32, tag="warm", name="warm", bufs=1)
                for _ in range(12):
                    nc.tensor.matmul(wps[:, :], lhsT=wk_sb[:, 0:128],
                                     rhs=wk_sb[:, 0:512], start=True, stop=True)
                run_all(proj_chunk(pps, "pp", "k", 0, 0))
                run_all(proj_chunk(pps, "pp", "q", 0, 0))
                run_all(vproj_st(pps, "pp", 0))
                run_all(vproj_st(pps, "pp", 1))

            # ---- SDPA + interleaved fillers ----
            with tc.tile_pool(name="scp", bufs=2, space="PSUM") as scp, \
                 tc.tile_pool(name="cdp", bufs=1, space="PSUM") as cdp, \
                 tc.tile_pool(name="dnp", bufs=1, space="PSUM") as dnp, \
                 tc.tile_pool(name="axp", bufs=1, space="PSUM") as axp, \
                 tc.tile_pool(name="ldr", bufs=2, space="DRAM") as ldr:

                # micro-filler FIFOs per (qh, pair-subloop); parts of one aux
                # chunk stay contiguous (aux ring=1), base work interleaves
                # between parts at step boundaries
                def kp(p, sc):
                    return proj_chunk(axp, "aux", "k", p, sc)

                def qp(p, sc):
                    return proj_chunk(axp, "aux", "q", p, sc)

                def vp(st):
                    return vproj_st(axp, "aux", st)

                def op(st, dch):
                    return outproj_chunk(axp, "aux", st, dch)

                F = {}
                F[(0, 0)] = (vp(2) + kp(0, 1) + vp(3) + vp(4) + kp(0, 2)
                             + vp(5) + vp(6) + kp(0, 3) + vp(7) + vp(8)
                             + vp(9) + vp(10) + kp(1, 0) + vp(11) + vp(12)
                             + qp(1, 0) + vp(13) + vp(14) + vp(15))
                F[(0, 1)] = kp(1, 1) + kp(1, 2) + kp(1, 3) + qp(0, 1) + qp(1, 1)
                for qh in range(1, QC):
                    s0 = (qh - 1) * 4
                    F[(qh, 0)] = (op(s0, 0) + op(s0, 1)
                                  + op(s0 + 1, 0) + op(s0 + 1, 1))
                    F[(qh, 1)] = (op(s0 + 2, 0) + op(s0 + 2, 1)
                                  + op(s0 + 3, 0) + op(s0 + 3, 1))
                    if qh < 3:
                        F[(qh, 1)] = F[(qh, 1)] + qp(0, qh + 1) + qp(1, qh + 1)
                POPN = {(0, 0): 4, (0, 1): 2, (1, 0): 1, (1, 1): 2,
                        (2, 0): 1, (2, 1): 2, (3, 0): 1, (3, 1): 1}

                for qh in range(QC):
                    qs = slice(qh * 512, (qh + 1) * 512)
                    ctx_ps = [cdp.tile([128, 512], f32, tag=f"ctx{p}",
                                       name=f"ctx{p}") for p in range(2)]
                    den_ps = dnp.tile([128, 512], f32, tag="den", name="den")
                    # unwritten partitions must stay finite for the reciprocal
                    # (only rows 0,32,64,96 are consumed downstream)
                    nc.vector.memset(den_ps[:, :], 1.0)
                    linv = nrm.tile([128, 512], f32, tag="linv", name="linv")
                    lbc = [nrm.tile([128, 512], f32, tag=f"lbc{p}",
                                    name=f"lbc{p}") for p in range(2)]

                    def scores_exp(p, ki):
                        s_ps = scp.tile([128, 1024], f32, tag="s", name="s")
                        nc.tensor.matmul(
                            s_ps[:, 0:512],
                            lhsT=kr_sb[p][0:64, ki * 128:(ki + 1) * 128],
                            rhs=qr_sb[p][0:64, qs],
                            tile_position=(0, 0), start=True, stop=True)
                        nc.tensor.matmul(
                            s_ps[:, 512:1024],
                            lhsT=kr_sb[p][64:128, ki * 128:(ki + 1) * 128],
                            rhs=qr_sb[p][64:128, qs],
                            tile_position=(64, 0), start=True, stop=True)
                        if p == 0:
                            e_t = e0_sb[:, ki * 1024:(ki + 1) * 1024]
                        else:
                            e_t = esp.tile([128, 1024], bf16, tag="e1", name="e1")
                        nc.scalar.activation(e_t, s_ps[:, :], Exp)
                        return e_t

                    def emit_norm(p):
                        # per-pair normalize; pair 0's is emitted mid-p1-loop
                        # so its reciprocal hides under the p1 exp stream
                        nc.vector.reciprocal(linv[p * 64:(p + 1) * 64, :],
                                             den_ps[p * 64:(p + 1) * 64, :])
                        lscr = ldr.tile([2, 512], f32, tag=f"lscr{p}",
                                        name=f"lscr{p}")
                        nc.sync.dma_start(lscr[:, :],
                                          linv[p * 64:(p + 1) * 64:32, :])
                        for half in range(2):
                            nc.sync.dma_start(
                                lbc[p][half * 64:(half + 1) * 64, :],
                                lscr[half:half + 1, :].partition_broadcast(64))
                        nc.vector.tensor_tensor(
                            ctxn_sb[p][:, qs], ctx_ps[p][:, :], lbc[p][:, :],
                            mult)

                    for p in range(2):
                        fifo = collections.deque(F[(qh, p)])
                        popn = POPN[(qh, p)]
                        e_cur = scores_exp(p, 0)
                        for ki in range(ST):
                            e_this = e_cur
                            if ki < ST - 1:
                                e_cur = scores_exp(p, ki + 1)
                            for h2 in range(2):
                                vo = ki * E + (2 * p + h2) * 64
                                nc.tensor.matmul(
                                    ctx_ps[p][h2 * 64:(h2 + 1) * 64, :],
                                    lhsT=v_sb[:, vo:vo + 64],
                                    rhs=e_this[:, h2 * 512:(h2 + 1) * 512],
                                    tile_position=(0, h2 * 64),
                                    start=(ki == 0), stop=(ki == ST - 1),
                                    skip_group_check=True)
                            # dens for pair p's two heads, fed by this e tile
                            for half in range(2):
                                g = p * 2 + half
                                nc.tensor.matmul(
                                    den_ps[g * 32:g * 32 + 1, :],
                                    lhsT=ones_sb[:, :],
                                    rhs=e_this[:, half * 512:(half + 1) * 512],
                                    tile_position=(0, g * 32),
                                    start=(ki == 0), stop=(ki == ST - 1),
                                    skip_group_check=True)
                            for _ in range(popn):
                                if fifo:
                                    fifo.popleft()()
                            if p == 1 and ki == 4:
                                emit_norm(0)
                        while fifo:
                            fifo.popleft()()
                    emit_norm(1)

                # epilogue: last qh's out-projection, ping-pong on scores slots
                for c in range(8):
                    run_all(outproj_chunk(scp, "s", 12 + c // 2, c % 2))

    _split_multiwait_instructions(nc, mybir, bass_rust.SyncInfo)
    return nc


def kernel(x_q, x_kv, wq, bq, wk, bk, wv, bv, wo, bo):
    from concourse import bass_utils

    x_q = np.asarray(x_q, dtype=np.float32)
    x_kv = np.asarray(x_kv, dtype=np.float32)
    wq = np.asarray(wq, dtype=np.float32); bq = np.asarray(bq, dtype=np.float32)
    wk = np.asarray(wk, dtype=np.float32); bk = np.asarray(bk, dtype=np.float32)
    wv = np.asarray(wv, dtype=np.float32); bv = np.asarray(bv, dtype=np.float32)
    wo = np.asarray(wo, dtype=np.float32); bo = np.asarray(bo, dtype=np.float32)

    with_bias = bool(bq.any() or bk.any() or bv.any())
    in_maps = _host_prep(x_q, x_kv, wq, bq, wk, bk, wv, bv, wo, with_bias)

    key = f"prog_{with_bias}"
    if key not in _PROGRAM_CACHE:
        _PROGRAM_CACHE[key] = build_program(with_bias)
    nc = _PROGRAM_CACHE[key]

    res = bass_utils.run_bass_kernel_spmd(
        nc, in_maps, core_ids=list(range(N_CORES)),
        trace=os.environ.get("KERNEL_TRACE", "") == "1")
    _PROGRAM_CACHE["last_result"] = res

    outp = np.zeros((B, S, D), dtype=np.float32)
    for c in range(N_CORES):
        outp[c // QUADS] += res.results[c]["out"]
    if bo.any():
        outp += bo[None, None, :]
    return outp


# revision 22
# speedup vs baseline: 1.2050x; 1.2050x over previous
"""Trainium2 Bass kernel for nn_CrossAttentionBlock (B=2, S=2048, D=1024, H=16, HD=64).

Sharding: 8 cores = 2 batches x 4 head-quads (4 heads each, E=256 channels).
Each core computes q/k/v projections for its quad, RoPE, SDPA, and a partial
output projection [S, D]; host sums the 4 partials per batch and adds bo.

v2 pipeline (vs the phase-serial baseline):
  - The SDPA inner loop is software-pipelined: scores(ki+1) is emitted before
    ctx(ki) so the PE never head-of-line blocks on the ScalarE exp.  ScalarE
    does nothing but exp (the hard ~147us floor); all PSUM->SBUF copies are on
    DVE, and the projections / output projection are interleaved into the
    exp-bound window as PE filler work.
  - Pair-major ki loop: pair 0's exp tiles are kept in a persistent SBUF ring
    so softmax denominators run as 4-way col-tiled concurrent matmuls in pair
    1's loop.  PSUM: scores ping-pong 4 banks + ctx 2 + den 1 + aux 1 = 8.
  - Inputs are host-packed so each s-chunk is a single DMA; the DMA order is
    prioritized so scores/exp start ~17us in instead of ~40us.
  - RoPE uses a host-permuted sin table: u = q*sinP read straight from PSUM,
    partition-swap of u via SBUF-SBUF DMA, one DVE add.  No ScalarE copies.
"""
import collections
import os
import sys

sys.path.insert(0, "/opt/trn_rl_repo")

import numpy as np
import ml_dtypes

BF16 = ml_dtypes.bfloat16

B, S, D, H = 2, 2048, 1024, 16
HD = D // H          # 64
DIM = HD // 2        # 32
QUADS = 4
E = D // QUADS       # 256 channels per core
ROPE_BASE = 10000.0
N_CORES = 8

KT = D // 128        # 8 d k-tiles
ST = S // 128        # 16 s-tiles
QC = S // 512        # 4 s-chunks
CW = KT * 512        # 4096 packed x cols per s-chunk


def _pack_x(xT):
    """[D(+1), S] d-major -> [128, QC*CW] s-chunk-major packed layout.

    packed[p, sc*CW + k*512 + s] = xT[k*128 + p, sc*512 + s]
    (bias row, if present, is returned separately)
    """
    xmain = xT[:D].reshape(KT, 128, QC, 512)
    packed = xmain.transpose(1, 2, 0, 3).reshape(128, QC * CW)
    return np.ascontiguousarray(packed)


def _pack_w(wT):
    """[D(+1), E] -> [128, KT*E]: packed[p, k*E + c] = wT[k*128 + p, c]."""
    return np.ascontiguousarray(
        wT[:D].reshape(KT, 128, E).transpose(1, 0, 2).reshape(128, KT * E))


def _host_prep(x_q, x_kv, wq, bq, wk, bk, wv, bv, wo, with_bias):
    perm = np.concatenate([np.arange(0, HD, 2), np.arange(1, HD, 2)])  # even|odd
    scale = 1.0 / np.sqrt(HD)

    freqs = np.exp(-np.arange(DIM, dtype=np.float64) * np.log(ROPE_BASE) / DIM)
    ang = np.arange(S, dtype=np.float64)[:, None] * freqs[None, :]     # [S, 32]
    cos = np.cos(ang).T                                                # [32, S]
    sin = np.sin(ang).T
    # rot-half: dst = q*cos64 + swap(q)*sin64, cos64=[cos;cos], sin64=[-sin;sin]
    # permuted-sin trick: ship sinP = swap_rows(sin64) = [sin;-sin]; then
    # u = q*sinP and swap(u) = swap(q)*sin64 exactly.
    cos64 = np.concatenate([cos, cos], axis=0)                         # [64, S]
    sinp64 = np.concatenate([sin, -sin], axis=0)
    cosT = np.concatenate([cos64, cos64], axis=0).astype(BF16)         # [128, S]
    sinPT = np.concatenate([sinp64, sinp64], axis=0).astype(BF16)

    def proj_mat(w, b, permute, s):
        blocks, brows = [], []
        for h in range(4):
            rows = slice(h * HD, (h + 1) * HD)
            wb_ = w[rows, :]
            bb = b[rows]
            if permute:
                wb_ = wb_[perm, :]
                bb = bb[perm]
            blocks.append(wb_ * s)
            brows.append(bb * s)
        wstack = np.concatenate(blocks, axis=0)          # [256, 1024]
        bstack = np.concatenate(brows, axis=0)           # [256]
        return wstack.T, bstack                          # [1024, 256], [256]

    ones_row = np.ones((1, S), dtype=BF16)
    in_maps = []
    for c in range(N_CORES):
        b_ = c // QUADS
        g = c % QUADS
        hs = slice(g * E, (g + 1) * E)
        wqT_, bq_ = proj_mat(wq[hs, :], bq[hs], True, scale)
        wkT_, bk_ = proj_mat(wk[hs, :], bk[hs], True, 1.0)
        wvT_, bv_ = proj_mat(wv[hs, :], bv[hs], False, 1.0)
        m = {
            "xqT": _pack_x(x_q[b_].T.astype(BF16)),
            "xkvT": _pack_x(x_kv[b_].T.astype(BF16)),
            "wqT": _pack_w(wqT_.astype(BF16)),
            "wkT": _pack_w(wkT_.astype(BF16)),
            "wvT": _pack_w(wvT_.astype(BF16)),
            "woT": np.ascontiguousarray(
                wo[:, hs].T.reshape(2, 128, D).transpose(1, 0, 2).reshape(128, 2 * D)
            ).astype(BF16),
            "cosT": np.ascontiguousarray(cosT),
            "sinPT": np.ascontiguousarray(sinPT),
            "ones_col": np.ones((128, 1), dtype=BF16),
        }
        if with_bias:
            m["xqb"] = ones_row.copy()
            m["xkvb"] = ones_row.copy()
            m["wqb"] = bq_[None, :].astype(BF16)
            m["wkb"] = bk_[None, :].astype(BF16)
            m["wvb"] = bv_[None, :].astype(BF16)
        in_maps.append(m)
    return in_maps


# ---------------------------------------------------------------------------
_PROGRAM_CACHE = {}


def _fixed_tile_context(tile_mod, bass_rust_mod, vector_clock_mod):
    """TileContext whose tail drain splits multi-sem waits into single-wait
    NOPs (this walrus rejects >1 sync-wait on one instruction)."""
    SyncInfo = bass_rust_mod.SyncInfo
    ScopedClock = vector_clock_mod.ScopedClock

    class TC(tile_mod.TileContext):
        def _drain_and_barrier(self, tick_clock, wait_clock):
            harvest = self.nc.sync.nop(nofuse=True)
            wait_clock.add_sem_waits(
                harvest.ins, ScopedClock({None: tick_clock.global_clock}))
            si = harvest.ins.sync_info
            waits = list(si.on_wait) if si is not None else []
            if len(waits) > 1:
                harvest.ins.sync_info = SyncInfo(
                    on_wait=[waits[0]], on_update=list(si.on_update))
                for w in waits[1:]:
                    nop = self.nc.sync.nop(nofuse=True)
                    nop.ins.sync_info = SyncInfo(on_wait=[w], on_update=[])
            self.nc.sync.drain()
            self.nc.all_engine_barrier()
            assert self.sems is not None
            popped = self.nc._tile_sem_poison_stack.pop()
            assert popped is self._sem_poison
            self.nc.clear_and_free_semaphores(list(self.sems.allocated().values()))
            self.nc.all_engine_barrier()

    return TC


def _split_multiwait_instructions(nc, mybir, SyncInfo):
    """This walrus build rejects >1 sync-wait per instruction; hoist extra
    waits onto single-wait NOPs inserted just before, on the same engine."""
    ctr = 0
    for blk in nc.m.functions[0].blocks:
        insts = blk.instructions
        i = 0
        while i < len(insts):
            inst = insts[i]
            si = inst.sync_info
            if si is not None and len(si.on_wait) > 1:
                waits = list(si.on_wait)
                inst.sync_info = SyncInfo(on_wait=[waits[-1]],
                                          on_update=list(si.on_update))
                nops = []
                for w in waits[:-1]:
                    nop = mybir.InstNoOp(name=f"waitsplit_{ctr}", ins=[], outs=[])
                    ctr += 1
                    nop.engine = inst.engine
                    nop.sync_info = SyncInfo(on_wait=[w], on_update=[])
                    nops.append(nop)
                insts[i:i] = nops
                i += len(nops)
            i += 1
    return ctr


def build_program(with_bias=False):
    import concourse.bass as bass
    import concourse.mybir as mybir
    import concourse.tile as tile
    import bass_rust
    from concourse import vector_clock

    f32 = mybir.dt.float32
    bf16 = mybir.dt.bfloat16
    Exp = mybir.ActivationFunctionType.Exp
    mult = mybir.AluOpType.mult
    add = mybir.AluOpType.add
    divide = mybir.AluOpType.divide

    nc = bass.Bass("TRN2", target_bir_lowering=False, debug=False,
                   num_devices=N_CORES)

    xqT = nc.dram_tensor("xqT", [128, QC * CW], bf16, kind="ExternalInput").ap()
    xkvT = nc.dram_tensor("xkvT", [128, QC * CW], bf16, kind="ExternalInput").ap()
    wqT = nc.dram_tensor("wqT", [128, KT * E], bf16, kind="ExternalInput").ap()
    wkT = nc.dram_tensor("wkT", [128, KT * E], bf16, kind="ExternalInput").ap()
    wvT = nc.dram_tensor("wvT", [128, KT * E], bf16, kind="ExternalInput").ap()
    woT = nc.dram_tensor("woT", [128, 2 * D], bf16, kind="ExternalInput").ap()
    cosT = nc.dram_tensor("cosT", [128, S], bf16, kind="ExternalInput").ap()
    sinPT = nc.dram_tensor("sinPT", [128, S], bf16, kind="ExternalInput").ap()
    ones_col = nc.dram_tensor("ones_col", [128, 1], bf16, kind="ExternalInput").ap()
    if with_bias:
        xqb = nc.dram_tensor("xqb", [1, S], bf16, kind="ExternalInput").ap()
        xkvb = nc.dram_tensor("xkvb", [1, S], bf16, kind="ExternalInput").ap()
        wqb = nc.dram_tensor("wqb", [1, E], bf16, kind="ExternalInput").ap()
        wkb = nc.dram_tensor("wkb", [1, E], bf16, kind="ExternalInput").ap()
        wvb = nc.dram_tensor("wvb", [1, E], bf16, kind="ExternalInput").ap()
    out = nc.dram_tensor("out", [S, D], f32, kind="ExternalOutput").ap()

    TC = _fixed_tile_context(tile, bass_rust, vector_clock)

    with TC(nc) as tc:
        with tc.tile_pool(name="per", bufs=1) as per, \
             tc.tile_pool(name="rp", bufs=2) as rp, \
             tc.tile_pool(name="esb", bufs=3) as esp, \
             tc.tile_pool(name="osb", bufs=2) as osb, \
             tc.tile_pool(name="nrm", bufs=2) as nrm:

            # ---- persistent tiles ----
            xq_sb = per.tile([128, QC * CW], bf16, tag="xq", name="xq")
            xkv_sb = per.tile([128, QC * CW], bf16, tag="xkv", name="xkv")
            wq_sb = per.tile([128, KT * E], bf16, tag="wq", name="wq")
            wk_sb = per.tile([128, KT * E], bf16, tag="wk", name="wk")
            wv_sb = per.tile([128, KT * E], bf16, tag="wv", name="wv")
            wo_sb = per.tile([128, 2 * D], bf16, tag="wo", name="wo")
            cos_sb = per.tile([128, S], bf16, tag="cos", name="cos")
            sinp_sb = per.tile([128, S], bf16, tag="sinp", name="sinp")
            ones_sb = per.tile([128, 1], bf16, tag="ones", name="ones")
            qr_sb = [per.tile([128, S], bf16, tag=f"qr{p}", name=f"qr{p}")
                     for p in range(2)]
            kr_sb = [per.tile([128, S], bf16, tag=f"kr{p}", name=f"kr{p}")
                     for p in range(2)]
            v_sb = per.tile([128, ST * E], bf16, tag="v", name="v")
            ctxn_sb = [per.tile([128, S], bf16, tag=f"ctxn{p}", name=f"ctxn{p}")
                       for p in range(2)]
            # pair-0 exp tiles for one qh, kept so dens run in pair 1's loop
            e0_sb = per.tile([128, ST * 1024], bf16, tag="e0", name="e0")
            ew_sb = per.tile([1, 1], f32, tag="ew", name="ew")
            onesf_sb = per.tile([128, 512], f32, tag="onesf", name="onesf")
            nc.vector.memset(onesf_sb[:, :], 1.0)
            if with_bias:
                xqb_sb = per.tile([1, S], bf16, tag="xqb", name="xqb")
                xkvb_sb = per.tile([1, S], bf16, tag="xkvb", name="xkvb")
                wqb_sb = per.tile([1, E], bf16, tag="wqb", name="wqb")
                wkb_sb = per.tile([1, E], bf16, tag="wkb", name="wkb")
                wvb_sb = per.tile([1, E], bf16, tag="wvb", name="wvb")

            # ---- input DMAs, priority order (critical path first) ----
            nc.sync.dma_start(wk_sb[:, :], wkT[:, :])
            nc.sync.dma_start(xkv_sb[:, 0:CW], xkvT[:, 0:CW])
            nc.sync.dma_start(wq_sb[:, :], wqT[:, :])
            nc.sync.dma_start(cos_sb[:, :], cosT[:, :])
            nc.sync.dma_start(sinp_sb[:, :], sinPT[:, :])
            nc.sync.dma_start(xq_sb[:, 0:CW], xqT[:, 0:CW])
            nc.sync.dma_start(wv_sb[:, :], wvT[:, :])
            if with_bias:
                nc.sync.dma_start(xkvb_sb[:, :], xkvb[:, :])
                nc.sync.dma_start(xqb_sb[:, :], xqb[:, :])
                nc.sync.dma_start(wkb_sb[:, :], wkb[:, :])
                nc.sync.dma_start(wqb_sb[:, :], wqb[:, :])
                nc.sync.dma_start(wvb_sb[:, :], wvb[:, :])
            for sc in range(1, QC):
                nc.sync.dma_start(xkv_sb[:, sc * CW:(sc + 1) * CW],
                                  xkvT[:, sc * CW:(sc + 1) * CW])
            for sc in range(1, QC):
                nc.sync.dma_start(xq_sb[:, sc * CW:(sc + 1) * CW],
                                  xqT[:, sc * CW:(sc + 1) * CW])
            nc.sync.dma_start(wo_sb[:, :], woT[:, :])
            nc.sync.dma_start(ones_sb[:, :], ones_col[:, :])

            # preload the exp table set during the DMA window
            nc.scalar.activation(ew_sb[0:1, 0:1], wk_sb[0:1, 0:1], Exp)

            # ---- helpers (each returns a list of ~0.7us micro-closures) ----
            def rope_chunk(q_ps, dst, sc):
                # dst = q*cos + swap32(q*sinP); swap32 of u is 4 SBUF-SBUF
                # DMAs issued on the idle GpSimd SWDGE queue (keeps both the
                # DVE and the Sync input-DMA stream clear)
                scs = slice(sc * 512, (sc + 1) * 512)
                u = rp.tile([128, 512], bf16, tag="u", name="u")
                usw = rp.tile([128, 512], bf16, tag="usw", name="usw")
                a = rp.tile([128, 512], bf16, tag="a", name="a")
                nc.vector.tensor_tensor(u[:, :], q_ps[:, :], sinp_sb[:, scs], mult)
                nc.vector.tensor_tensor(a[:, :], q_ps[:, :], cos_sb[:, scs], mult)
                for d0, s0 in ((0, 32), (32, 0), (64, 96), (96, 64)):
                    nc.gpsimd.dma_start(usw[d0:d0 + 32, :], u[s0:s0 + 32, :])
                nc.vector.tensor_tensor(dst[:, scs], a[:, :], usw[:, :], add)

            def proj_chunk(pool, tag, which, p, sc):
                if which == "q":
                    w_, src, dst = wq_sb, xq_sb, qr_sb
                    wb_, srcb_ = (wqb_sb, xqb_sb) if with_bias else (None, None)
                else:
                    w_, src, dst = wk_sb, xkv_sb, kr_sb
                    wb_, srcb_ = (wkb_sb, xkvb_sb) if with_bias else (None, None)
                cell = {}

                def mm_part(k0, k1):
                    if k0 == 0:
                        cell["ps"] = pool.tile([128, 512], f32, tag=tag,
                                               name=f"{which}proj")
                    q_ps = cell["ps"]
                    for k in range(k0, k1):
                        for h2 in range(2):
                            co = k * E + p * 128 + h2 * 64
                            nc.tensor.matmul(
                                q_ps[h2 * 64:(h2 + 1) * 64, :],
                                lhsT=w_[:, co:co + 64],
                                rhs=src[:, sc * CW + k * 512:
                                        sc * CW + (k + 1) * 512],
                                tile_position=(0, h2 * 64),
                                start=(k == 0),
                                stop=(k == KT - 1) and not with_bias,
                                skip_group_check=True)
                    if k1 == KT and with_bias:
                        nc.tensor.matmul(
                            q_ps[:, :], lhsT=wb_[:, p * 128:(p + 1) * 128],
                            rhs=srcb_[:, sc * 512:(sc + 1) * 512],
                            start=False, stop=True, skip_group_check=True)

                return [lambda k0=k0: mm_part(k0, k0 + 2) for k0 in (0, 2, 4, 6)] \
                    + [lambda: rope_chunk(cell["ps"], dst[p], sc)]

            def vproj_st(pool, tag, st):
                sc, si = st // 4, st % 4
                cell = {}

                def mm_part(k0, k1):
                    if k0 == 0:
                        cell["ps"] = pool.tile([128, E], f32, tag=tag,
                                               name="vproj")
                    v_ps = cell["ps"]
                    for k in range(k0, k1):
                        for h2 in range(2):
                            co = sc * CW + k * 512 + si * 128 + h2 * 64
                            nc.tensor.matmul(
                                v_ps[h2 * 64:(h2 + 1) * 64, :],
                                lhsT=xkv_sb[:, co:co + 64],
                                rhs=wv_sb[:, k * E:(k + 1) * E],
                                tile_position=(0, h2 * 64),
                                start=(k == 0),
                                stop=(k == KT - 1) and not with_bias,
                                skip_group_check=True)
                    if k1 == KT:
                        if with_bias:
                            nc.tensor.matmul(
                                v_ps[:, :],
                                lhsT=xkvb_sb[:, st * 128:(st + 1) * 128],
                                rhs=wvb_sb[:, :],
                                start=False, stop=True, skip_group_check=True)
                        nc.vector.tensor_copy(
                            v_sb[:, st * E:(st + 1) * E], v_ps[:, :])

                return [lambda: mm_part(0, 4), lambda: mm_part(4, KT)]

            def outproj_chunk(pool, tag, st, dch):
                cell = {}

                def mm_part(p):
                    if p == 0:
                        cell["ps"] = pool.tile([128, 512], f32, tag=tag,
                                               name="oproj")
                    o_ps = cell["ps"]
                    for h2 in range(2):
                        nc.tensor.matmul(
                            o_ps[h2 * 64:(h2 + 1) * 64, :],
                            lhsT=ctxn_sb[p][:, st * 128 + h2 * 64:
                                            st * 128 + h2 * 64 + 64],
                            rhs=wo_sb[:, p * D + dch * 512:
                                      p * D + (dch + 1) * 512],
                            tile_position=(0, h2 * 64),
                            start=(p == 0), stop=(p == 1),
                            skip_group_check=True)
                    if p == 1:
                        o_t = osb.tile([128, 512], f32, tag="o", name="o")
                        nc.vector.tensor_copy(o_t[:, :], o_ps[:, :])
                        nc.sync.dma_start(
                            out[st * 128:(st + 1) * 128,
                                dch * 512:(dch + 1) * 512], o_t[:, :])

                return [lambda: mm_part(0), lambda: mm_part(1)]

            def run_all(parts):
                for f in parts:
                    f()

            # ---- lead-in (own PSUM pool, released before SDPA) ----
            with tc.tile_pool(name="pps", bufs=2, space="PSUM") as pps:
                wps = pps.tile([128, 512], f32, tag="warm", name="warm", bufs=1)
                for _ in range(3):
                    nc.tensor.matmul(wps[:, :], lhsT=wk_sb[:, 0:128],
                                     rhs=wk_sb[:, 0:512], start=True, stop=True)
                run_all(proj_chunk(pps, "pp", "k", 0, 0))
                run_all(proj_chunk(pps, "pp", "q", 0, 0))
                run_all(vproj_st(pps, "pp", 0))
                run_all(vproj_st(pps, "pp", 1))

            # ---- SDPA + interleaved fillers ----
            with tc.tile_pool(name="scp", bufs=2, space="PSUM") as scp, \
                 tc.tile_pool(name="cdp", bufs=1, space="PSUM") as cdp, \
                 tc.tile_pool(name="dnp", bufs=1, space="PSUM") as dnp, \
                 tc.tile_pool(name="axp", bufs=1, space="PSUM") as axp, \
                 tc.tile_pool(name="ldr", bufs=2, space="DRAM") as ldr:

                # micro-filler FIFOs per (qh, pair-subloop); parts of one aux
                # chunk stay contiguous (aux ring=1), base work interleaves
                # between parts at step boundaries
                def kp(p, sc):
                    return proj_chunk(axp, "aux", "k", p, sc)

                def qp(p, sc):
                    return proj_chunk(axp, "aux", "q", p, sc)

                def vp(st):
                    return vproj_st(axp, "aux", st)

                def op(st, dch):
                    return outproj_chunk(axp, "aux", st, dch)

                F = {}
                F[(0, 0)] = (vp(2) + kp(0, 1) + vp(3) + vp(4) + kp(0, 2)
                             + vp(5) + vp(6) + kp(0, 3) + vp(7) + vp(8)
                             + vp(9) + vp(10) + kp(1, 0) + vp(11) + vp(12)
                             + qp(1, 0) + vp(13) + vp(14) + vp(15))
                F[(0, 1)] = kp(1, 1) + kp(1, 2) + kp(1, 3) + qp(0, 1) + qp(1, 1)
                for qh in range(1, QC):
                    s0 = (qh - 1) * 4
                    F[(qh, 0)] = (op(s0, 0) + op(s0, 1)
                                  + op(s0 + 1, 0) + op(s0 + 1, 1))
                    F[(qh, 1)] = (op(s0 + 2, 0) + op(s0 + 2, 1)
                                  + op(s0 + 3, 0) + op(s0 + 3, 1))
                    if qh < 3:
                        F[(qh, 1)] = F[(qh, 1)] + qp(0, qh + 1) + qp(1, qh + 1)
                POPN = {(0, 0): 4, (0, 1): 2, (1, 0): 1, (1, 1): 2,
                        (2, 0): 1, (2, 1): 2, (3, 0): 1, (3, 1): 1}

                for qh in range(QC):
                    qs = slice(qh * 512, (qh + 1) * 512)
                    ctx_ps = [cdp.tile([128, 512], f32, tag=f"ctx{p}",
                                       name=f"ctx{p}") for p in range(2)]
                    den_ps = dnp.tile([128, 512], f32, tag="den", name="den")
                    # unwritten partitions must stay finite for the reciprocal
                    # (only rows 0,32,64,96 are consumed downstream)
                    nc.vector.memset(den_ps[:, :], 1.0)
                    linv = nrm.tile([128, 512], f32, tag="linv", name="linv")
                    lbc = [nrm.tile([128, 512], f32, tag=f"lbc{p}",
                                    name=f"lbc{p}") for p in range(2)]

                    def scores_exp(p, ki):
                        s_ps = scp.tile([128, 1024], f32, tag="s", name="s")
                        nc.tensor.matmul(
                            s_ps[:, 0:512],
                            lhsT=kr_sb[p][0:64, ki * 128:(ki + 1) * 128],
                            rhs=qr_sb[p][0:64, qs],
                            tile_position=(0, 0), start=True, stop=True)
                        nc.tensor.matmul(
                            s_ps[:, 512:1024],
                            lhsT=kr_sb[p][64:128, ki * 128:(ki + 1) * 128],
                            rhs=qr_sb[p][64:128, qs],
                            tile_position=(64, 0), start=True, stop=True)
                        if p == 0:
                            e_t = e0_sb[:, ki * 1024:(ki + 1) * 1024]
                        else:
                            e_t = esp.tile([128, 1024], bf16, tag="e1", name="e1")
                        nc.scalar.activation(e_t, s_ps[:, :], Exp)
                        return e_t

                    def emit_norm():
                        nc.vector.reciprocal(linv[:, :], den_ps[:, :])
                        lscr = ldr.tile([4, 512], f32, tag="lscr", name="lscr")
                        nc.sync.dma_start(lscr[:, :], linv[0:128:32, :])
                        for g, (p, half) in enumerate(
                                ((0, 0), (0, 1), (1, 0), (1, 1))):
                            nc.sync.dma_start(
                                lbc[p][half * 64:(half + 1) * 64, :],
                                lscr[g:g + 1, :].partition_broadcast(64))
                        for p in range(2):
                            nc.vector.tensor_tensor(
                                ctxn_sb[p][:, qs], ctx_ps[p][:, :],
                                lbc[p][:, :], mult)

                    for p in range(2):
                        fifo = collections.deque(F[(qh, p)])
                        popn = POPN[(qh, p)]
                        e_cur = scores_exp(p, 0)
                        for ki in range(ST):
                            e_this = e_cur
                            if ki < ST - 1:
                                e_cur = scores_exp(p, ki + 1)
                            for h2 in range(2):
                                vo = ki * E + (2 * p + h2) * 64
                                nc.tensor.matmul(
                                    ctx_ps[p][h2 * 64:(h2 + 1) * 64, :],
                                    lhsT=v_sb[:, vo:vo + 64],
                                    rhs=e_this[:, h2 * 512:(h2 + 1) * 512],
                                    tile_position=(0, h2 * 64),
                                    start=(ki == 0), stop=(ki == ST - 1),
                                    skip_group_check=True)
                            # dens for pair p's two heads, fed by this e tile
                            for half in range(2):
                                g = p * 2 + half
                                nc.tensor.matmul(
                                    den_ps[g * 32:g * 32 + 1, :],
                                    lhsT=ones_sb[:, :],
                                    rhs=e_this[:, half * 512:(half + 1) * 512],
                                    tile_position=(0, g * 32),
                                    start=(ki == 0), stop=(ki == ST - 1),
                                    skip_group_check=True)
                            for _ in range(popn):
                                if fifo:
                                    fifo.popleft()()
                        while fifo:
                            fifo.popleft()()
                    emit_norm()

                # epilogue: last qh's out-projection, ping-pong on scores slots
                for c in range(8):
                    run_all(outproj_chunk(scp, "s", 12 + c // 2, c % 2))

    _split_multiwait_instructions(nc, mybir, bass_rust.SyncInfo)
    return nc


def kernel(x_q, x_kv, wq, bq, wk, bk, wv, bv, wo, bo):
    from concourse import bass_utils

    x_q = np.asarray(x_q, dtype=np.float32)
    x_kv = np.asarray(x_kv, dtype=np.float32)
    wq = np.asarray(wq, dtype=np.float32); bq = np.asarray(bq, dtype=np.float32)
    wk = np.asarray(wk, dtype=np.float32); bk = np.asarray(bk, dtype=np.float32)
    wv = np.asarray(wv, dtype=np.float32); bv = np.asarray(bv, dtype=np.float32)
    wo = np.asarray(wo, dtype=np.float32); bo = np.asarray(bo, dtype=np.float32)

    with_bias = bool(bq.any() or bk.any() or bv.any())
    in_maps = _host_prep(x_q, x_kv, wq, bq, wk, bk, wv, bv, wo, with_bias)

    key = f"prog_{with_bias}"
    if key not in _PROGRAM_CACHE:
        _PROGRAM_CACHE[key] = build_program(with_bias)
    nc = _PROGRAM_CACHE[key]

    res = bass_utils.run_bass_kernel_spmd(
        nc, in_maps, core_ids=list(range(N_CORES)),
        trace=os.environ.get("KERNEL_TRACE", "") == "1")
    _PROGRAM_CACHE["last_result"] = res

    outp = np.zeros((B, S, D), dtype=np.float32)
    for c in range(N_CORES):
        outp[c // QUADS] += res.results[c]["out"]
    if bo.any():
        outp += bo[None, None, :]
    return outp


# revision 27
# speedup vs baseline: 1.2504x; 1.0377x over previous
"""Trainium2 Bass kernel for nn_CrossAttentionBlock (B=2, S=2048, D=1024, H=16, HD=64).

Sharding: 8 cores = 2 batches x 4 head-quads (4 heads each, E=256 channels).
Each core computes q/k/v projections for its quad, RoPE, SDPA, and a partial
output projection [S, D]; host sums the 4 partials per batch and adds bo.

v2 pipeline (vs the phase-serial baseline):
  - The SDPA inner loop is software-pipelined: scores(ki+1) is emitted before
    ctx(ki) so the PE never head-of-line blocks on the ScalarE exp.  ScalarE
    does nothing but exp (the hard ~147us floor); all PSUM->SBUF copies are on
    DVE, and the projections / output projection are interleaved into the
    exp-bound window as PE filler work.
  - Pair-major ki loop: pair 0's exp tiles are kept in a persistent SBUF ring
    so softmax denominators run as 4-way col-tiled concurrent matmuls in pair
    1's loop.  PSUM: scores ping-pong 4 banks + ctx 2 + den 1 + aux 1 = 8.
  - Inputs are host-packed so each s-chunk is a single DMA; the DMA order is
    prioritized so scores/exp start ~17us in instead of ~40us.
  - RoPE uses a host-permuted sin table: u = q*sinP read straight from PSUM,
    partition-swap of u via SBUF-SBUF DMA, one DVE add.  No ScalarE copies.
"""
import collections
import os
import sys

sys.path.insert(0, "/opt/trn_rl_repo")

import numpy as np
import ml_dtypes

BF16 = ml_dtypes.bfloat16

B, S, D, H = 2, 2048, 1024, 16
HD = D // H          # 64
DIM = HD // 2        # 32
QUADS = 4
E = D // QUADS       # 256 channels per core
ROPE_BASE = 10000.0
N_CORES = 8

KT = D // 128        # 8 d k-tiles
ST = S // 128        # 16 s-tiles
QC = S // 512        # 4 s-chunks
CW = KT * 512        # 4096 packed x cols per s-chunk


def _pack_x(xT):
    """[D(+1), S] d-major -> [128, QC*CW] s-chunk-major packed layout.

    packed[p, sc*CW + k*512 + s] = xT[k*128 + p, sc*512 + s]
    (bias row, if present, is returned separately)
    """
    xmain = xT[:D].reshape(KT, 128, QC, 512)
    packed = xmain.transpose(1, 2, 0, 3).reshape(128, QC * CW)
    return np.ascontiguousarray(packed)


def _pack_w(wT):
    """[D(+1), E] -> [128, KT*E]: packed[p, k*E + c] = wT[k*128 + p, c]."""
    return np.ascontiguousarray(
        wT[:D].reshape(KT, 128, E).transpose(1, 0, 2).reshape(128, KT * E))


def _host_prep(x_q, x_kv, wq, bq, wk, bk, wv, bv, wo, with_bias):
    perm = np.concatenate([np.arange(0, HD, 2), np.arange(1, HD, 2)])  # even|odd
    scale = 1.0 / np.sqrt(HD)

    freqs = np.exp(-np.arange(DIM, dtype=np.float64) * np.log(ROPE_BASE) / DIM)
    ang = np.arange(S, dtype=np.float64)[:, None] * freqs[None, :]     # [S, 32]
    cos = np.cos(ang).T                                                # [32, S]
    sin = np.sin(ang).T
    # rot-half: dst = q*cos64 + swap(q)*sin64, cos64=[cos;cos], sin64=[-sin;sin]
    # permuted-sin trick: ship sinP = swap_rows(sin64) = [sin;-sin]; then
    # u = q*sinP and swap(u) = swap(q)*sin64 exactly.
    cos64 = np.concatenate([cos, cos], axis=0)                         # [64, S]
    sinp64 = np.concatenate([sin, -sin], axis=0)
    cosT = np.concatenate([cos64, cos64], axis=0).astype(BF16)         # [128, S]
    sinPT = np.concatenate([sinp64, sinp64], axis=0).astype(BF16)

    def proj_mat(w, b, permute, s):
        blocks, brows = [], []
        for h in range(4):
            rows = slice(h * HD, (h + 1) * HD)
            wb_ = w[rows, :]
            bb = b[rows]
            if permute:
                wb_ = wb_[perm, :]
                bb = bb[perm]
            blocks.append(wb_ * s)
            brows.append(bb * s)
        wstack = np.concatenate(blocks, axis=0)          # [256, 1024]
        bstack = np.concatenate(brows, axis=0)           # [256]
        return wstack.T, bstack                          # [1024, 256], [256]

    ones_row = np.ones((1, S), dtype=BF16)
    in_maps = []
    for c in range(N_CORES):
        b_ = c // QUADS
        g = c % QUADS
        hs = slice(g * E, (g + 1) * E)
        wqT_, bq_ = proj_mat(wq[hs, :], bq[hs], True, scale)
        wkT_, bk_ = proj_mat(wk[hs, :], bk[hs], True, 1.0)
        wvT_, bv_ = proj_mat(wv[hs, :], bv[hs], False, 1.0)
        m = {
            "xqT": _pack_x(x_q[b_].T.astype(BF16)),
            "xkvT": _pack_x(x_kv[b_].T.astype(BF16)),
            "wqT": _pack_w(wqT_.astype(BF16)),
            "wkT": _pack_w(wkT_.astype(BF16)),
            "wvT": _pack_w(wvT_.astype(BF16)),
            "woT": np.ascontiguousarray(
                wo[:, hs].T.reshape(2, 128, D).transpose(1, 0, 2).reshape(128, 2 * D)
            ).astype(BF16),
            "cosT": np.ascontiguousarray(cosT),
            "sinPT": np.ascontiguousarray(sinPT),
            "ones_col": np.ones((128, 1), dtype=BF16),
        }
        if with_bias:
            m["xqb"] = ones_row.copy()
            m["xkvb"] = ones_row.copy()
            m["wqb"] = bq_[None, :].astype(BF16)
            m["wkb"] = bk_[None, :].astype(BF16)
            m["wvb"] = bv_[None, :].astype(BF16)
        in_maps.append(m)
    return in_maps


# ---------------------------------------------------------------------------
_PROGRAM_CACHE = {}


def _fixed_tile_context(tile_mod, bass_rust_mod, vector_clock_mod):
    """TileContext whose tail drain splits multi-sem waits into single-wait
    NOPs (this walrus rejects >1 sync-wait on one instruction)."""
    SyncInfo = bass_rust_mod.SyncInfo
    ScopedClock = vector_clock_mod.ScopedClock

    class TC(tile_mod.TileContext):
        def _drain_and_barrier(self, tick_clock, wait_clock):
            harvest = self.nc.sync.nop(nofuse=True)
            wait_clock.add_sem_waits(
                harvest.ins, ScopedClock({None: tick_clock.global_clock}))
            si = harvest.ins.sync_info
            waits = list(si.on_wait) if si is not None else []
            if len(waits) > 1:
                harvest.ins.sync_info = SyncInfo(
                    on_wait=[waits[0]], on_update=list(si.on_update))
                for w in waits[1:]:
                    nop = self.nc.sync.nop(nofuse=True)
                    nop.ins.sync_info = SyncInfo(on_wait=[w], on_update=[])
            self.nc.sync.drain()
            self.nc.all_engine_barrier()
            assert self.sems is not None
            popped = self.nc._tile_sem_poison_stack.pop()
            assert popped is self._sem_poison
            self.nc.clear_and_free_semaphores(list(self.sems.allocated().values()))
            self.nc.all_engine_barrier()

    return TC


def _split_multiwait_instructions(nc, mybir, SyncInfo):
    """This walrus build rejects >1 sync-wait per instruction; hoist extra
    waits onto single-wait NOPs inserted just before, on the same engine."""
    ctr = 0
    for blk in nc.m.functions[0].blocks:
        insts = blk.instructions
        i = 0
        while i < len(insts):
            inst = insts[i]
            si = inst.sync_info
            if si is not None and len(si.on_wait) > 1:
                waits = list(si.on_wait)
                inst.sync_info = SyncInfo(on_wait=[waits[-1]],
                                          on_update=list(si.on_update))
                nops = []
                for w in waits[:-1]:
                    nop = mybir.InstNoOp(name=f"waitsplit_{ctr}", ins=[], outs=[])
                    ctr += 1
                    nop.engine = inst.engine
                    nop.sync_info = SyncInfo(on_wait=[w], on_update=[])
                    nops.append(nop)
                insts[i:i] = nops
                i += len(nops)
            i += 1
    return ctr


def build_program(with_bias=False):
    import concourse.bass as bass
    import concourse.mybir as mybir
    import concourse.tile as tile
    import bass_rust
    from concourse import vector_clock

    f32 = mybir.dt.float32
    bf16 = mybir.dt.bfloat16
    Exp = mybir.ActivationFunctionType.Exp
    mult = mybir.AluOpType.mult
    add = mybir.AluOpType.add
    divide = mybir.AluOpType.divide

    nc = bass.Bass("TRN2", target_bir_lowering=False, debug=False,
                   num_devices=N_CORES)

    xqT = nc.dram_tensor("xqT", [128, QC * CW], bf16, kind="ExternalInput").ap()
    xkvT = nc.dram_tensor("xkvT", [128, QC * CW], bf16, kind="ExternalInput").ap()
    wqT = nc.dram_tensor("wqT", [128, KT * E], bf16, kind="ExternalInput").ap()
    wkT = nc.dram_tensor("wkT", [128, KT * E], bf16, kind="ExternalInput").ap()
    wvT = nc.dram_tensor("wvT", [128, KT * E], bf16, kind="ExternalInput").ap()
    woT = nc.dram_tensor("woT", [128, 2 * D], bf16, kind="ExternalInput").ap()
    cosT = nc.dram_tensor("cosT", [128, S], bf16, kind="ExternalInput").ap()
    sinPT = nc.dram_tensor("sinPT", [128, S], bf16, kind="ExternalInput").ap()
    ones_col = nc.dram_tensor("ones_col", [128, 1], bf16, kind="ExternalInput").ap()
    if with_bias:
        xqb = nc.dram_tensor("xqb", [1, S], bf16, kind="ExternalInput").ap()
        xkvb = nc.dram_tensor("xkvb", [1, S], bf16, kind="ExternalInput").ap()
        wqb = nc.dram_tensor("wqb", [1, E], bf16, kind="ExternalInput").ap()
        wkb = nc.dram_tensor("wkb", [1, E], bf16, kind="ExternalInput").ap()
        wvb = nc.dram_tensor("wvb", [1, E], bf16, kind="ExternalInput").ap()
    out = nc.dram_tensor("out", [S, D], f32, kind="ExternalOutput").ap()

    TC = _fixed_tile_context(tile, bass_rust, vector_clock)

    with TC(nc) as tc:
        with tc.tile_pool(name="per", bufs=1) as per, \
             tc.tile_pool(name="rp", bufs=2) as rp, \
             tc.tile_pool(name="esb", bufs=3) as esp, \
             tc.tile_pool(name="osb", bufs=2) as osb, \
             tc.tile_pool(name="nrm", bufs=2) as nrm:

            # ---- persistent tiles ----
            xq_sb = per.tile([128, QC * CW], bf16, tag="xq", name="xq")
            xkv_sb = per.tile([128, QC * CW], bf16, tag="xkv", name="xkv")
            wq_sb = per.tile([128, KT * E], bf16, tag="wq", name="wq")
            wk_sb = per.tile([128, KT * E], bf16, tag="wk", name="wk")
            wv_sb = per.tile([128, KT * E], bf16, tag="wv", name="wv")
            wo_sb = per.tile([128, 2 * D], bf16, tag="wo", name="wo")
            cos_sb = per.tile([128, S], bf16, tag="cos", name="cos")
            sinp_sb = per.tile([128, S], bf16, tag="sinp", name="sinp")
            ones_sb = per.tile([128, 1], bf16, tag="ones", name="ones")
            qr_sb = [per.tile([128, S], bf16, tag=f"qr{p}", name=f"qr{p}")
                     for p in range(2)]
            kr_sb = [per.tile([128, S], bf16, tag=f"kr{p}", name=f"kr{p}")
                     for p in range(2)]
            v_sb = per.tile([128, ST * E], bf16, tag="v", name="v")
            ctxn_sb = [per.tile([128, S], bf16, tag=f"ctxn{p}", name=f"ctxn{p}")
                       for p in range(2)]
            # pair-0 exp tiles for one qh, kept so dens run in pair 1's loop
            e0_sb = per.tile([128, ST * 1024], bf16, tag="e0", name="e0")
            ew_sb = per.tile([1, 1], f32, tag="ew", name="ew")
            onesf_sb = per.tile([128, 512], f32, tag="onesf", name="onesf")
            nc.vector.memset(onesf_sb[:, :], 1.0)
            if with_bias:
                xqb_sb = per.tile([1, S], bf16, tag="xqb", name="xqb")
                xkvb_sb = per.tile([1, S], bf16, tag="xkvb", name="xkvb")
                wqb_sb = per.tile([1, E], bf16, tag="wqb", name="wqb")
                wkb_sb = per.tile([1, E], bf16, tag="wkb", name="wkb")
                wvb_sb = per.tile([1, E], bf16, tag="wvb", name="wvb")

            # ---- input DMAs, priority order (critical path first) ----
            nc.sync.dma_start(wk_sb[:, :], wkT[:, :])
            nc.sync.dma_start(xkv_sb[:, 0:CW // 2], xkvT[:, 0:CW // 2])
            nc.sync.dma_start(xkv_sb[:, CW // 2:CW], xkvT[:, CW // 2:CW])
            nc.sync.dma_start(wq_sb[:, :], wqT[:, :])
            nc.sync.dma_start(cos_sb[:, :], cosT[:, :])
            nc.sync.dma_start(sinp_sb[:, :], sinPT[:, :])
            nc.sync.dma_start(xq_sb[:, 0:CW // 2], xqT[:, 0:CW // 2])
            nc.sync.dma_start(xq_sb[:, CW // 2:CW], xqT[:, CW // 2:CW])
            nc.sync.dma_start(wv_sb[:, :], wvT[:, :])
            if with_bias:
                nc.sync.dma_start(xkvb_sb[:, :], xkvb[:, :])
                nc.sync.dma_start(xqb_sb[:, :], xqb[:, :])
                nc.sync.dma_start(wkb_sb[:, :], wkb[:, :])
                nc.sync.dma_start(wqb_sb[:, :], wqb[:, :])
                nc.sync.dma_start(wvb_sb[:, :], wvb[:, :])
            for sc in range(1, QC):
                nc.sync.dma_start(xkv_sb[:, sc * CW:(sc + 1) * CW],
                                  xkvT[:, sc * CW:(sc + 1) * CW])
            for sc in range(1, QC):
                nc.sync.dma_start(xq_sb[:, sc * CW:(sc + 1) * CW],
                                  xqT[:, sc * CW:(sc + 1) * CW])
            nc.sync.dma_start(wo_sb[:, :], woT[:, :])
            nc.sync.dma_start(ones_sb[:, :], ones_col[:, :])

            # preload the exp table set during the DMA window
            nc.scalar.activation(ew_sb[0:1, 0:1], wk_sb[0:1, 0:1], Exp)

            # ---- helpers (each returns a list of ~0.7us micro-closures) ----
            def rope_chunk(q_ps, dst, sc):
                # dst = q*cos + swap32(q*sinP); swap32 of u is 4 SBUF-SBUF
                # DMAs issued on the idle GpSimd SWDGE queue (keeps both the
                # DVE and the Sync input-DMA stream clear)
                scs = slice(sc * 512, (sc + 1) * 512)
                u = rp.tile([128, 512], bf16, tag="u", name="u")
                usw = rp.tile([128, 512], bf16, tag="usw", name="usw")
                a = rp.tile([128, 512], bf16, tag="a", name="a")
                nc.vector.tensor_tensor(u[:, :], q_ps[:, :], sinp_sb[:, scs], mult)
                nc.vector.tensor_tensor(a[:, :], q_ps[:, :], cos_sb[:, scs], mult)
                for d0, s0 in ((0, 32), (32, 0), (64, 96), (96, 64)):
                    nc.sync.dma_start(usw[d0:d0 + 32, :], u[s0:s0 + 32, :])
                nc.vector.tensor_tensor(dst[:, scs], a[:, :], usw[:, :], add)

            def proj_chunk(pool, tag, which, p, sc):
                if which == "q":
                    w_, src, dst = wq_sb, xq_sb, qr_sb
                    wb_, srcb_ = (wqb_sb, xqb_sb) if with_bias else (None, None)
                else:
                    w_, src, dst = wk_sb, xkv_sb, kr_sb
                    wb_, srcb_ = (wkb_sb, xkvb_sb) if with_bias else (None, None)
                cell = {}

                def mm_part(k0, k1):
                    if k0 == 0:
                        cell["ps"] = pool.tile([128, 512], f32, tag=tag,
                                               name=f"{which}proj")
                    q_ps = cell["ps"]
                    for k in range(k0, k1):
                        for h2 in range(2):
                            co = k * E + p * 128 + h2 * 64
                            nc.tensor.matmul(
                                q_ps[h2 * 64:(h2 + 1) * 64, :],
                                lhsT=w_[:, co:co + 64],
                                rhs=src[:, sc * CW + k * 512:
                                        sc * CW + (k + 1) * 512],
                                tile_position=(0, h2 * 64),
                                start=(k == 0),
                                stop=(k == KT - 1) and not with_bias,
                                skip_group_check=True)
                    if k1 == KT and with_bias:
                        nc.tensor.matmul(
                            q_ps[:, :], lhsT=wb_[:, p * 128:(p + 1) * 128],
                            rhs=srcb_[:, sc * 512:(sc + 1) * 512],
                            start=False, stop=True, skip_group_check=True)

                return [lambda k0=k0: mm_part(k0, k0 + 2) for k0 in (0, 2, 4, 6)] \
                    + [lambda: rope_chunk(cell["ps"], dst[p], sc)]

            def vproj_st(pool, tag, st):
                sc, si = st // 4, st % 4
                cell = {}

                def mm_part(k0, k1):
                    if k0 == 0:
                        cell["ps"] = pool.tile([128, E], f32, tag=tag,
                                               name="vproj")
                    v_ps = cell["ps"]
                    for k in range(k0, k1):
                        for h2 in range(2):
                            co = sc * CW + k * 512 + si * 128 + h2 * 64
                            nc.tensor.matmul(
                                v_ps[h2 * 64:(h2 + 1) * 64, :],
                                lhsT=xkv_sb[:, co:co + 64],
                                rhs=wv_sb[:, k * E:(k + 1) * E],
                                tile_position=(0, h2 * 64),
                                start=(k == 0),
                                stop=(k == KT - 1) and not with_bias,
                                skip_group_check=True)
                    if k1 == KT:
                        if with_bias:
                            nc.tensor.matmul(
                                v_ps[:, :],
                                lhsT=xkvb_sb[:, st * 128:(st + 1) * 128],
                                rhs=wvb_sb[:, :],
                                start=False, stop=True, skip_group_check=True)
                        nc.vector.tensor_copy(
                            v_sb[:, st * E:(st + 1) * E], v_ps[:, :])

                return [lambda: mm_part(0, 4), lambda: mm_part(4, KT)]

            def outproj_chunk(pool, tag, st, dch):
                cell = {}

                def mm_part(p):
                    if p == 0:
                        cell["ps"] = pool.tile([128, 512], f32, tag=tag,
                                               name="oproj")
                    o_ps = cell["ps"]
                    for h2 in range(2):
                        nc.tensor.matmul(
                            o_ps[h2 * 64:(h2 + 1) * 64, :],
                            lhsT=ctxn_sb[p][:, st * 128 + h2 * 64:
                                            st * 128 + h2 * 64 + 64],
                            rhs=wo_sb[:, p * D + dch * 512:
                                      p * D + (dch + 1) * 512],
                            tile_position=(0, h2 * 64),
                            start=(p == 0), stop=(p == 1),
                            skip_group_check=True)
                    if p == 1:
                        o_t = osb.tile([128, 512], f32, tag="o", name="o")
                        nc.vector.tensor_copy(o_t[:, :], o_ps[:, :])
                        nc.sync.dma_start(
                            out[st * 128:(st + 1) * 128,
                                dch * 512:(dch + 1) * 512], o_t[:, :])

                return [lambda: mm_part(0), lambda: mm_part(1)]

            def run_all(parts):
                for f in parts:
                    f()

            # ---- lead-in (own PSUM pool, released before SDPA) ----
            with tc.tile_pool(name="pps", bufs=2, space="PSUM") as pps:
                wps = pps.tile([128, 512], f32, tag="warm", name="warm", bufs=1)
                for _ in range(6):
                    nc.tensor.matmul(wps[:, 0:256], lhsT=wk_sb[:, 0:128],
                                     rhs=wk_sb[:, 0:256], start=True, stop=True)
                # kproj/qproj MM parts first, both rope tails after, so the
                # two DVE rope chains overlap each other and the qproj MMs
                kparts = proj_chunk(pps, "pp", "k", 0, 0)
                qparts = proj_chunk(pps, "pp", "q", 0, 0)
                for f in kparts[:4]:
                    f()
                for f in qparts[:4]:
                    f()
                kparts[4]()
                qparts[4]()
                run_all(vproj_st(pps, "pp", 0))
                run_all(vproj_st(pps, "pp", 1))

            # ---- SDPA + interleaved fillers ----
            with tc.tile_pool(name="scp", bufs=2, space="PSUM") as scp, \
                 tc.tile_pool(name="cdp", bufs=1, space="PSUM") as cdp, \
                 tc.tile_pool(name="dnp", bufs=1, space="PSUM") as dnp, \
                 tc.tile_pool(name="axp", bufs=1, space="PSUM") as axp, \
                 tc.tile_pool(name="ldr", bufs=2, space="DRAM") as ldr:

                # micro-filler FIFOs per (qh, pair-subloop); parts of one aux
                # chunk stay contiguous (aux ring=1), base work interleaves
                # between parts at step boundaries
                def kp(p, sc):
                    return proj_chunk(axp, "aux", "k", p, sc)

                def qp(p, sc):
                    return proj_chunk(axp, "aux", "q", p, sc)

                def vp(st):
                    return vproj_st(axp, "aux", st)

                def op(st, dch):
                    return outproj_chunk(axp, "aux", st, dch)

                F = {}
                F[(0, 0)] = (vp(2) + kp(0, 1) + vp(3) + vp(4) + kp(0, 2)
                             + vp(5) + vp(6) + kp(0, 3) + vp(7) + vp(8)
                             + vp(9) + vp(10) + kp(1, 0) + vp(11) + vp(12)
                             + qp(1, 0) + vp(13) + vp(14) + vp(15))
                F[(0, 1)] = kp(1, 1) + kp(1, 2) + kp(1, 3) + qp(0, 1) + qp(1, 1)
                for qh in range(1, QC):
                    s0 = (qh - 1) * 4
                    # den-free p0 subloop carries the out-projection; p1 only
                    # the next q-projection chunk
                    F[(qh, 0)] = (op(s0, 0) + op(s0, 1) + op(s0 + 1, 0)
                                  + op(s0 + 1, 1) + op(s0 + 2, 0)
                                  + op(s0 + 2, 1) + op(s0 + 3, 0)
                                  + op(s0 + 3, 1))
                    F[(qh, 1)] = (qp(0, qh + 1) + qp(1, qh + 1)) if qh < 3 else []
                POPN = {(0, 0): 4, (0, 1): 2, (1, 0): 1, (1, 1): 1,
                        (2, 0): 1, (2, 1): 1, (3, 0): 1, (3, 1): 1}

                for qh in range(QC):
                    qs = slice(qh * 512, (qh + 1) * 512)
                    ctx_ps = [cdp.tile([128, 512], f32, tag=f"ctx{p}",
                                       name=f"ctx{p}") for p in range(2)]
                    den_ps = dnp.tile([128, 512], f32, tag="den", name="den")
                    # unwritten partitions must stay finite for the reciprocal
                    # (only rows 0,32,64,96 are consumed downstream)
                    nc.vector.memset(den_ps[:, :], 1.0)
                    linv = nrm.tile([128, 512], f32, tag="linv", name="linv")
                    lbc = [nrm.tile([128, 512], f32, tag=f"lbc{p}",
                                    name=f"lbc{p}") for p in range(2)]

                    def scores_exp(p, ki):
                        s_ps = scp.tile([128, 1024], f32, tag="s", name="s")
                        nc.tensor.matmul(
                            s_ps[:, 0:512],
                            lhsT=kr_sb[p][0:64, ki * 128:(ki + 1) * 128],
                            rhs=qr_sb[p][0:64, qs],
                            tile_position=(0, 0), start=True, stop=True)
                        nc.tensor.matmul(
                            s_ps[:, 512:1024],
                            lhsT=kr_sb[p][64:128, ki * 128:(ki + 1) * 128],
                            rhs=qr_sb[p][64:128, qs],
                            tile_position=(64, 0), start=True, stop=True)
                        if p == 0:
                            e_t = e0_sb[:, ki * 1024:(ki + 1) * 1024]
                        else:
                            e_t = esp.tile([128, 1024], bf16, tag="e1", name="e1")
                        nc.scalar.activation(e_t, s_ps[:, :], Exp)
                        return e_t

                    def emit_norm():
                        nc.vector.reciprocal(linv[:, :], den_ps[:, :])
                        lscr = ldr.tile([4, 512], f32, tag="lscr", name="lscr")
                        nc.sync.dma_start(lscr[:, :], linv[0:128:32, :])
                        for g, (p, half) in enumerate(
                                ((0, 0), (0, 1), (1, 0), (1, 1))):
                            nc.sync.dma_start(
                                lbc[p][half * 64:(half + 1) * 64, :],
                                lscr[g:g + 1, :].partition_broadcast(64))
                        for p in range(2):
                            nc.vector.tensor_tensor(
                                ctxn_sb[p][:, qs], ctx_ps[p][:, :],
                                lbc[p][:, :], mult)

                    for p in range(2):
                        fifo = collections.deque(F[(qh, p)])
                        popn = POPN[(qh, p)]
                        e_cur = scores_exp(p, 0)
                        for ki in range(ST):
                            e_this = e_cur
                            if ki < ST - 1:
                                e_cur = scores_exp(p, ki + 1)
                            for h2 in range(2):
                                vo = ki * E + (2 * p + h2) * 64
                                nc.tensor.matmul(
                                    ctx_ps[p][h2 * 64:(h2 + 1) * 64, :],
                                    lhsT=v_sb[:, vo:vo + 64],
                                    rhs=e_this[:, h2 * 512:(h2 + 1) * 512],
                                    tile_position=(0, h2 * 64),
                                    start=(ki == 0), stop=(ki == ST - 1),
                                    skip_group_check=True)
                            if p == 1:
                                # all four dens 4-way col-concurrent, using the
                                # stored pair-0 e tiles plus this fresh one
                                for g, (pp_, half) in enumerate(
                                        ((0, 0), (0, 1), (1, 0), (1, 1))):
                                    src = (e0_sb[:, ki * 1024:(ki + 1) * 1024]
                                           if pp_ == 0 else e_this)
                                    nc.tensor.matmul(
                                        den_ps[g * 32:g * 32 + 1, :],
                                        lhsT=ones_sb[:, :],
                                        rhs=src[:, half * 512:(half + 1) * 512],
                                        tile_position=(0, g * 32),
                                        start=(ki == 0), stop=(ki == ST - 1),
                                        skip_group_check=True)
                            for _ in range(popn):
                                if fifo:
                                    fifo.popleft()()
                        while fifo:
                            fifo.popleft()()
                    emit_norm()

                # epilogue: last qh's out-projection, ping-pong on scores slots
                for c in range(8):
                    run_all(outproj_chunk(scp, "s", 12 + c // 2, c % 2))

    _split_multiwait_instructions(nc, mybir, bass_rust.SyncInfo)
    return nc


def kernel(x_q, x_kv, wq, bq, wk, bk, wv, bv, wo, bo):
    from concourse import bass_utils

    x_q = np.asarray(x_q, dtype=np.float32)
    x_kv = np.asarray(x_kv, dtype=np.float32)
    wq = np.asarray(wq, dtype=np.float32); bq = np.asarray(bq, dtype=np.float32)
    wk = np.asarray(wk, dtype=np.float32); bk = np.asarray(bk, dtype=np.float32)
    wv = np.asarray(wv, dtype=np.float32); bv = np.asarray(bv, dtype=np.float32)
    wo = np.asarray(wo, dtype=np.float32); bo = np.asarray(bo, dtype=np.float32)

    with_bias = bool(bq.any() or bk.any() or bv.any())
    in_maps = _host_prep(x_q, x_kv, wq, bq, wk, bk, wv, bv, wo, with_bias)

    key = f"prog_{with_bias}"
    if key not in _PROGRAM_CACHE:
        _PROGRAM_CACHE[key] = build_program(with_bias)
    nc = _PROGRAM_CACHE[key]

    res = bass_utils.run_bass_kernel_spmd(
        nc, in_maps, core_ids=list(range(N_CORES)),
        trace=os.environ.get("KERNEL_TRACE", "") == "1")
    _PROGRAM_CACHE["last_result"] = res

    outp = np.zeros((B, S, D), dtype=np.float32)
    for c in range(N_CORES):
        outp[c // QUADS] += res.results[c]["out"]
    if bo.any():
        outp += bo[None, None, :]
    return outp


# revision 32
# speedup vs baseline: 1.2900x; 1.0317x over previous
"""Trainium2 Bass kernel for nn_CrossAttentionBlock (B=2, S=2048, D=1024, H=16, HD=64).

Sharding: 8 cores = 2 batches x 4 head-quads (4 heads each, E=256 channels).
Each core computes q/k/v projections for its quad, RoPE, SDPA, and a partial
output projection [S, D]; host sums the 4 partials per batch and adds bo.

v2 pipeline (vs the phase-serial baseline):
  - The SDPA inner loop is software-pipelined: scores(ki+1) is emitted before
    ctx(ki) so the PE never head-of-line blocks on the ScalarE exp.  ScalarE
    does nothing but exp (the hard ~147us floor); all PSUM->SBUF copies are on
    DVE, and the projections / output projection are interleaved into the
    exp-bound window as PE filler work.
  - Pair-major ki loop: pair 0's exp tiles are kept in a persistent SBUF ring
    so softmax denominators run as 4-way col-tiled concurrent matmuls in pair
    1's loop.  PSUM: scores ping-pong 4 banks + ctx 2 + den 1 + aux 1 = 8.
  - Inputs are host-packed so each s-chunk is a single DMA; the DMA order is
    prioritized so scores/exp start ~17us in instead of ~40us.
  - RoPE uses a host-permuted sin table: u = q*sinP read straight from PSUM,
    partition-swap of u via SBUF-SBUF DMA, one DVE add.  No ScalarE copies.
"""
import collections
import os
import sys

sys.path.insert(0, "/opt/trn_rl_repo")

import numpy as np
import ml_dtypes

BF16 = ml_dtypes.bfloat16

B, S, D, H = 2, 2048, 1024, 16
HD = D // H          # 64
DIM = HD // 2        # 32
QUADS = 4
E = D // QUADS       # 256 channels per core
ROPE_BASE = 10000.0
N_CORES = 8

KT = D // 128        # 8 d k-tiles
ST = S // 128        # 16 s-tiles
QC = S // 512        # 4 s-chunks
CW = KT * 512        # 4096 packed x cols per s-chunk


def _pack_x(xT):
    """[D(+1), S] d-major -> [128, QC*CW] s-chunk-major packed layout.

    packed[p, sc*CW + k*512 + s] = xT[k*128 + p, sc*512 + s]
    (bias row, if present, is returned separately)
    """
    xmain = xT[:D].reshape(KT, 128, QC, 512)
    packed = xmain.transpose(1, 2, 0, 3).reshape(128, QC * CW)
    return np.ascontiguousarray(packed)


def _pack_w(wT):
    """[D(+1), E] -> [128, KT*E]: packed[p, k*E + c] = wT[k*128 + p, c]."""
    return np.ascontiguousarray(
        wT[:D].reshape(KT, 128, E).transpose(1, 0, 2).reshape(128, KT * E))


def _host_prep(x_q, x_kv, wq, bq, wk, bk, wv, bv, wo, with_bias):
    perm = np.concatenate([np.arange(0, HD, 2), np.arange(1, HD, 2)])  # even|odd
    scale = 1.0 / np.sqrt(HD)

    freqs = np.exp(-np.arange(DIM, dtype=np.float64) * np.log(ROPE_BASE) / DIM)
    ang = np.arange(S, dtype=np.float64)[:, None] * freqs[None, :]     # [S, 32]
    cos = np.cos(ang).T                                                # [32, S]
    sin = np.sin(ang).T
    # rot-half: dst = q*cos64 + swap(q)*sin64, cos64=[cos;cos], sin64=[-sin;sin]
    # permuted-sin trick: ship sinP = swap_rows(sin64) = [sin;-sin]; then
    # u = q*sinP and swap(u) = swap(q)*sin64 exactly.
    cos64 = np.concatenate([cos, cos], axis=0)                         # [64, S]
    sinp64 = np.concatenate([sin, -sin], axis=0)
    cosT = np.concatenate([cos64, cos64], axis=0).astype(BF16)         # [128, S]
    sinPT = np.concatenate([sinp64, sinp64], axis=0).astype(BF16)

    def proj_mat(w, b, permute, s):
        blocks, brows = [], []
        for h in range(4):
            rows = slice(h * HD, (h + 1) * HD)
            wb_ = w[rows, :]
            bb = b[rows]
            if permute:
                wb_ = wb_[perm, :]
                bb = bb[perm]
            blocks.append(wb_ * s)
            brows.append(bb * s)
        wstack = np.concatenate(blocks, axis=0)          # [256, 1024]
        bstack = np.concatenate(brows, axis=0)           # [256]
        return wstack.T, bstack                          # [1024, 256], [256]

    ones_row = np.ones((1, S), dtype=BF16)
    in_maps = []
    for c in range(N_CORES):
        b_ = c // QUADS
        g = c % QUADS
        hs = slice(g * E, (g + 1) * E)
        wqT_, bq_ = proj_mat(wq[hs, :], bq[hs], True, scale)
        wkT_, bk_ = proj_mat(wk[hs, :], bk[hs], True, 1.0)
        wvT_, bv_ = proj_mat(wv[hs, :], bv[hs], False, 1.0)
        m = {
            "xqT": _pack_x(x_q[b_].T.astype(BF16)),
            "xkvT": _pack_x(x_kv[b_].T.astype(BF16)),
            "wqT": _pack_w(wqT_.astype(BF16)),
            "wkT": _pack_w(wkT_.astype(BF16)),
            "wvT": _pack_w(wvT_.astype(BF16)),
            "woT": np.ascontiguousarray(
                wo[:, hs].T.reshape(2, 128, D).transpose(1, 0, 2).reshape(128, 2 * D)
            ).astype(BF16),
            "cosT": np.ascontiguousarray(cosT),
            "sinPT": np.ascontiguousarray(sinPT),
            "ones_col": np.ones((128, 1), dtype=BF16),
        }
        if with_bias:
            m["xqb"] = ones_row.copy()
            m["xkvb"] = ones_row.copy()
            m["wqb"] = bq_[None, :].astype(BF16)
            m["wkb"] = bk_[None, :].astype(BF16)
            m["wvb"] = bv_[None, :].astype(BF16)
        in_maps.append(m)
    return in_maps


# ---------------------------------------------------------------------------
_PROGRAM_CACHE = {}


def _fixed_tile_context(tile_mod, bass_rust_mod, vector_clock_mod):
    """TileContext whose tail drain splits multi-sem waits into single-wait
    NOPs (this walrus rejects >1 sync-wait on one instruction)."""
    SyncInfo = bass_rust_mod.SyncInfo
    ScopedClock = vector_clock_mod.ScopedClock

    class TC(tile_mod.TileContext):
        def _drain_and_barrier(self, tick_clock, wait_clock):
            harvest = self.nc.sync.nop(nofuse=True)
            wait_clock.add_sem_waits(
                harvest.ins, ScopedClock({None: tick_clock.global_clock}))
            si = harvest.ins.sync_info
            waits = list(si.on_wait) if si is not None else []
            if len(waits) > 1:
                harvest.ins.sync_info = SyncInfo(
                    on_wait=[waits[0]], on_update=list(si.on_update))
                for w in waits[1:]:
                    nop = self.nc.sync.nop(nofuse=True)
                    nop.ins.sync_info = SyncInfo(on_wait=[w], on_update=[])
            self.nc.sync.drain()
            self.nc.all_engine_barrier()
            assert self.sems is not None
            popped = self.nc._tile_sem_poison_stack.pop()
            assert popped is self._sem_poison
            self.nc.clear_and_free_semaphores(list(self.sems.allocated().values()))
            self.nc.all_engine_barrier()

    return TC


def _split_multiwait_instructions(nc, mybir, SyncInfo):
    """This walrus build rejects >1 sync-wait per instruction; hoist extra
    waits onto single-wait NOPs inserted just before, on the same engine."""
    ctr = 0
    for blk in nc.m.functions[0].blocks:
        insts = blk.instructions
        i = 0
        while i < len(insts):
            inst = insts[i]
            si = inst.sync_info
            if si is not None and len(si.on_wait) > 1:
                waits = list(si.on_wait)
                inst.sync_info = SyncInfo(on_wait=[waits[-1]],
                                          on_update=list(si.on_update))
                nops = []
                for w in waits[:-1]:
                    nop = mybir.InstNoOp(name=f"waitsplit_{ctr}", ins=[], outs=[])
                    ctr += 1
                    nop.engine = inst.engine
                    nop.sync_info = SyncInfo(on_wait=[w], on_update=[])
                    nops.append(nop)
                insts[i:i] = nops
                i += len(nops)
            i += 1
    return ctr


def build_program(with_bias=False):
    import concourse.bass as bass
    import concourse.mybir as mybir
    import concourse.tile as tile
    import bass_rust
    from concourse import vector_clock

    f32 = mybir.dt.float32
    bf16 = mybir.dt.bfloat16
    Exp = mybir.ActivationFunctionType.Exp
    mult = mybir.AluOpType.mult
    add = mybir.AluOpType.add
    divide = mybir.AluOpType.divide

    nc = bass.Bass("TRN2", target_bir_lowering=False, debug=False,
                   num_devices=N_CORES)

    xqT = nc.dram_tensor("xqT", [128, QC * CW], bf16, kind="ExternalInput").ap()
    xkvT = nc.dram_tensor("xkvT", [128, QC * CW], bf16, kind="ExternalInput").ap()
    wqT = nc.dram_tensor("wqT", [128, KT * E], bf16, kind="ExternalInput").ap()
    wkT = nc.dram_tensor("wkT", [128, KT * E], bf16, kind="ExternalInput").ap()
    wvT = nc.dram_tensor("wvT", [128, KT * E], bf16, kind="ExternalInput").ap()
    woT = nc.dram_tensor("woT", [128, 2 * D], bf16, kind="ExternalInput").ap()
    cosT = nc.dram_tensor("cosT", [128, S], bf16, kind="ExternalInput").ap()
    sinPT = nc.dram_tensor("sinPT", [128, S], bf16, kind="ExternalInput").ap()
    ones_col = nc.dram_tensor("ones_col", [128, 1], bf16, kind="ExternalInput").ap()
    if with_bias:
        xqb = nc.dram_tensor("xqb", [1, S], bf16, kind="ExternalInput").ap()
        xkvb = nc.dram_tensor("xkvb", [1, S], bf16, kind="ExternalInput").ap()
        wqb = nc.dram_tensor("wqb", [1, E], bf16, kind="ExternalInput").ap()
        wkb = nc.dram_tensor("wkb", [1, E], bf16, kind="ExternalInput").ap()
        wvb = nc.dram_tensor("wvb", [1, E], bf16, kind="ExternalInput").ap()
    out = nc.dram_tensor("out", [S, D], f32, kind="ExternalOutput").ap()

    TC = _fixed_tile_context(tile, bass_rust, vector_clock)

    with TC(nc) as tc:
        with tc.tile_pool(name="per", bufs=1) as per, \
             tc.tile_pool(name="rp", bufs=2) as rp, \
             tc.tile_pool(name="esb", bufs=3) as esp, \
             tc.tile_pool(name="osb", bufs=2) as osb, \
             tc.tile_pool(name="nrm", bufs=2) as nrm:

            # ---- persistent tiles ----
            xq_sb = per.tile([128, QC * CW], bf16, tag="xq", name="xq")
            xkv_sb = per.tile([128, QC * CW], bf16, tag="xkv", name="xkv")
            wq_sb = per.tile([128, KT * E], bf16, tag="wq", name="wq")
            wk_sb = per.tile([128, KT * E], bf16, tag="wk", name="wk")
            wv_sb = per.tile([128, KT * E], bf16, tag="wv", name="wv")
            wo_sb = per.tile([128, 2 * D], bf16, tag="wo", name="wo")
            cos_sb = per.tile([128, S], bf16, tag="cos", name="cos")
            sinp_sb = per.tile([128, S], bf16, tag="sinp", name="sinp")
            ones_sb = per.tile([128, 1], bf16, tag="ones", name="ones")
            qr_sb = [per.tile([128, S], bf16, tag=f"qr{p}", name=f"qr{p}")
                     for p in range(2)]
            kr_sb = [per.tile([128, S], bf16, tag=f"kr{p}", name=f"kr{p}")
                     for p in range(2)]
            v_sb = per.tile([128, ST * E], bf16, tag="v", name="v")
            ctxn_sb = [per.tile([128, S], bf16, tag=f"ctxn{p}", name=f"ctxn{p}")
                       for p in range(2)]
            # pair-0 exp tiles for one qh, kept so dens run in pair 1's loop
            e0_sb = per.tile([128, ST * 1024], bf16, tag="e0", name="e0")
            ew_sb = per.tile([1, 1], f32, tag="ew", name="ew")
            onesf_sb = per.tile([128, 512], f32, tag="onesf", name="onesf")
            nc.vector.memset(onesf_sb[:, :], 1.0)
            gate_sb = per.tile([1, 8], bf16, tag="gate", name="gate")
            if with_bias:
                xqb_sb = per.tile([1, S], bf16, tag="xqb", name="xqb")
                xkvb_sb = per.tile([1, S], bf16, tag="xkvb", name="xkvb")
                wqb_sb = per.tile([1, E], bf16, tag="wqb", name="wqb")
                wkb_sb = per.tile([1, E], bf16, tag="wkb", name="wkb")
                wvb_sb = per.tile([1, E], bf16, tag="wvb", name="wvb")

            # ---- input DMAs, priority order (critical path first) ----
            nc.sync.dma_start(wk_sb[:, :], wkT[:, :])
            nc.sync.dma_start(xkv_sb[:, 0:CW // 2], xkvT[:, 0:CW // 2])
            nc.sync.dma_start(xkv_sb[:, CW // 2:CW], xkvT[:, CW // 2:CW])
            nc.sync.dma_start(wq_sb[:, :], wqT[:, :])
            nc.sync.dma_start(cos_sb[:, :], cosT[:, :])
            nc.sync.dma_start(sinp_sb[:, :], sinPT[:, :])
            nc.sync.dma_start(xq_sb[:, 0:CW // 2], xqT[:, 0:CW // 2])
            nc.sync.dma_start(xq_sb[:, CW // 2:CW], xqT[:, CW // 2:CW])
            nc.sync.dma_start(wv_sb[:, :], wvT[:, :])
            if with_bias:
                nc.sync.dma_start(xkvb_sb[:, :], xkvb[:, :])
                nc.sync.dma_start(xqb_sb[:, :], xqb[:, :])
                nc.sync.dma_start(wkb_sb[:, :], wkb[:, :])
                nc.sync.dma_start(wqb_sb[:, :], wqb[:, :])
                nc.sync.dma_start(wvb_sb[:, :], wvb[:, :])
            nc.sync.dma_start(ones_sb[:, :], ones_col[:, :])
            # bulk loads go on the GpSimd SWDGE queue, gated behind a copy
            # that depends on the last critical tile — keeps the Sync HW
            # queue shallow (rope-swap DMAs would otherwise head-of-line
            # block behind these transfers) and leaves critical-load HBM
            # bandwidth uncontended
            nc.gpsimd.tensor_copy(gate_sb[:, :], xq_sb[0:1, CW - 8:CW])
            for sc in range(1, QC):
                nc.gpsimd.dma_start(xkv_sb[:, sc * CW:(sc + 1) * CW],
                                    xkvT[:, sc * CW:(sc + 1) * CW])
            for sc in range(1, QC):
                nc.gpsimd.dma_start(xq_sb[:, sc * CW:(sc + 1) * CW],
                                    xqT[:, sc * CW:(sc + 1) * CW])
            nc.gpsimd.dma_start(wo_sb[:, :], woT[:, :])

            # preload the exp table set during the DMA window
            nc.scalar.activation(ew_sb[0:1, 0:1], wk_sb[0:1, 0:1], Exp)

            # ---- helpers (each returns a list of ~0.7us micro-closures) ----
            def rope_chunk(q_ps, dst, sc):
                # dst = q*cos + swap32(q*sinP); swap32 of u is 4 SBUF-SBUF
                # DMAs issued on the idle GpSimd SWDGE queue (keeps both the
                # DVE and the Sync input-DMA stream clear)
                scs = slice(sc * 512, (sc + 1) * 512)
                u = rp.tile([128, 512], bf16, tag="u", name="u")
                usw = rp.tile([128, 512], bf16, tag="usw", name="usw")
                a = rp.tile([128, 512], bf16, tag="a", name="a")
                nc.vector.tensor_tensor(u[:, :], q_ps[:, :], sinp_sb[:, scs], mult)
                nc.vector.tensor_tensor(a[:, :], q_ps[:, :], cos_sb[:, scs], mult)
                for d0, s0 in ((0, 32), (32, 0), (64, 96), (96, 64)):
                    nc.sync.dma_start(usw[d0:d0 + 32, :], u[s0:s0 + 32, :])
                nc.vector.tensor_tensor(dst[:, scs], a[:, :], usw[:, :], add)

            def proj_chunk(pool, tag, which, p, sc):
                if which == "q":
                    w_, src, dst = wq_sb, xq_sb, qr_sb
                    wb_, srcb_ = (wqb_sb, xqb_sb) if with_bias else (None, None)
                else:
                    w_, src, dst = wk_sb, xkv_sb, kr_sb
                    wb_, srcb_ = (wkb_sb, xkvb_sb) if with_bias else (None, None)
                cell = {}

                def mm_part(k0, k1):
                    if k0 == 0:
                        cell["ps"] = pool.tile([128, 512], f32, tag=tag,
                                               name=f"{which}proj")
                    q_ps = cell["ps"]
                    for k in range(k0, k1):
                        for h2 in range(2):
                            co = k * E + p * 128 + h2 * 64
                            nc.tensor.matmul(
                                q_ps[h2 * 64:(h2 + 1) * 64, :],
                                lhsT=w_[:, co:co + 64],
                                rhs=src[:, sc * CW + k * 512:
                                        sc * CW + (k + 1) * 512],
                                tile_position=(0, h2 * 64),
                                start=(k == 0),
                                stop=(k == KT - 1) and not with_bias,
                                skip_group_check=True)
                    if k1 == KT and with_bias:
                        nc.tensor.matmul(
                            q_ps[:, :], lhsT=wb_[:, p * 128:(p + 1) * 128],
                            rhs=srcb_[:, sc * 512:(sc + 1) * 512],
                            start=False, stop=True, skip_group_check=True)

                return [lambda k0=k0: mm_part(k0, k0 + 2) for k0 in (0, 2, 4, 6)] \
                    + [lambda: rope_chunk(cell["ps"], dst[p], sc)]

            def vproj_st(pool, tag, st):
                sc, si = st // 4, st % 4
                cell = {}

                def mm_part(k0, k1):
                    if k0 == 0:
                        cell["ps"] = pool.tile([128, E], f32, tag=tag,
                                               name="vproj")
                    v_ps = cell["ps"]
                    for k in range(k0, k1):
                        for h2 in range(2):
                            co = sc * CW + k * 512 + si * 128 + h2 * 64
                            nc.tensor.matmul(
                                v_ps[h2 * 64:(h2 + 1) * 64, :],
                                lhsT=xkv_sb[:, co:co + 64],
                                rhs=wv_sb[:, k * E:(k + 1) * E],
                                tile_position=(0, h2 * 64),
                                start=(k == 0),
                                stop=(k == KT - 1) and not with_bias,
                                skip_group_check=True)
                    if k1 == KT:
                        if with_bias:
                            nc.tensor.matmul(
                                v_ps[:, :],
                                lhsT=xkvb_sb[:, st * 128:(st + 1) * 128],
                                rhs=wvb_sb[:, :],
                                start=False, stop=True, skip_group_check=True)
                        nc.vector.tensor_copy(
                            v_sb[:, st * E:(st + 1) * E], v_ps[:, :])

                return [lambda: mm_part(0, 4), lambda: mm_part(4, KT)]

            def outproj_chunk(pool, tag, st, dch):
                cell = {}

                def mm_part(p):
                    if p == 0:
                        cell["ps"] = pool.tile([128, 512], f32, tag=tag,
                                               name="oproj")
                    o_ps = cell["ps"]
                    for h2 in range(2):
                        nc.tensor.matmul(
                            o_ps[h2 * 64:(h2 + 1) * 64, :],
                            lhsT=ctxn_sb[p][:, st * 128 + h2 * 64:
                                            st * 128 + h2 * 64 + 64],
                            rhs=wo_sb[:, p * D + dch * 512:
                                      p * D + (dch + 1) * 512],
                            tile_position=(0, h2 * 64),
                            start=(p == 0), stop=(p == 1),
                            skip_group_check=True)
                    if p == 1:
                        o_t = osb.tile([128, 512], f32, tag="o", name="o")
                        nc.vector.tensor_copy(o_t[:, :], o_ps[:, :])
                        nc.sync.dma_start(
                            out[st * 128:(st + 1) * 128,
                                dch * 512:(dch + 1) * 512], o_t[:, :])

                return [lambda: mm_part(0), lambda: mm_part(1)]

            def run_all(parts):
                for f in parts:
                    f()

            # ---- lead-in (own PSUM pool, released before SDPA) ----
            with tc.tile_pool(name="pps", bufs=2, space="PSUM") as pps:
                wps = pps.tile([128, 512], f32, tag="warm", name="warm", bufs=1)
                for _ in range(6):
                    nc.tensor.matmul(wps[:, 0:256], lhsT=wk_sb[:, 0:128],
                                     rhs=wk_sb[:, 0:256], start=True, stop=True)
                # kproj/qproj MM parts first, both rope tails after, so the
                # two DVE rope chains overlap each other and the qproj MMs
                kparts = proj_chunk(pps, "pp", "k", 0, 0)
                qparts = proj_chunk(pps, "pp", "q", 0, 0)
                for f in kparts[:4]:
                    f()
                for f in qparts[:4]:
                    f()
                kparts[4]()
                qparts[4]()
                run_all(vproj_st(pps, "pp", 0))
                run_all(vproj_st(pps, "pp", 1))

            # ---- SDPA + interleaved fillers ----
            with tc.tile_pool(name="scp", bufs=2, space="PSUM") as scp, \
                 tc.tile_pool(name="cdp", bufs=1, space="PSUM") as cdp, \
                 tc.tile_pool(name="dnp", bufs=1, space="PSUM") as dnp, \
                 tc.tile_pool(name="axp", bufs=1, space="PSUM") as axp, \
                 tc.tile_pool(name="ldr", bufs=2, space="DRAM") as ldr:

                # micro-filler FIFOs per (qh, pair-subloop); parts of one aux
                # chunk stay contiguous (aux ring=1), base work interleaves
                # between parts at step boundaries
                # p0-subloop fillers alternate between the aux bank and the
                # den bank (den_ps is only allocated at each p1 prologue, so
                # its bank is free during p0) — an effective ring of 2 that
                # lets chunk N+1's matmuls overlap chunk N's DVE copy.
                def kp(p, sc):
                    return lambda pool, tag: proj_chunk(pool, tag, "k", p, sc)

                def qp(p, sc):
                    return lambda pool, tag: proj_chunk(pool, tag, "q", p, sc)

                def vp(st):
                    return lambda pool, tag: vproj_st(pool, tag, st)

                def op(st, dch):
                    return lambda pool, tag: outproj_chunk(pool, tag, st, dch)

                def alt(chunks):
                    parts = []
                    for i, ch in enumerate(chunks):
                        pool, tag = (axp, "aux") if i % 2 == 0 else (dnp, "den")
                        parts += ch(pool, tag)
                    return parts

                def aux_only(chunks):
                    parts = []
                    for ch in chunks:
                        parts += ch(axp, "aux")
                    return parts

                F = {}
                F[(0, 0)] = alt([vp(2), kp(0, 1), vp(3), vp(4), kp(0, 2),
                                 vp(5), vp(6), kp(0, 3), vp(7), vp(8),
                                 vp(9), vp(10), kp(1, 0), vp(11), vp(12),
                                 qp(1, 0), vp(13), vp(14), vp(15)])
                F[(0, 1)] = aux_only([kp(1, 1), kp(1, 2), kp(1, 3),
                                      qp(0, 1), qp(1, 1)])
                for qh in range(1, QC):
                    s0 = (qh - 1) * 4
                    F[(qh, 0)] = alt([op(s0, 0), op(s0, 1), op(s0 + 1, 0),
                                      op(s0 + 1, 1), op(s0 + 2, 0),
                                      op(s0 + 2, 1), op(s0 + 3, 0),
                                      op(s0 + 3, 1)])
                    F[(qh, 1)] = (aux_only([qp(0, qh + 1), qp(1, qh + 1)])
                                  if qh < 3 else [])
                POPN = {(0, 0): 4, (0, 1): 2, (1, 0): 1, (1, 1): 1,
                        (2, 0): 1, (2, 1): 1, (3, 0): 1, (3, 1): 1}

                for qh in range(QC):
                    qs = slice(qh * 512, (qh + 1) * 512)
                    ctx_ps = [cdp.tile([128, 512], f32, tag=f"ctx{p}",
                                       name=f"ctx{p}") for p in range(2)]
                    den_ps = None  # allocated at the p1 prologue
                    linv = nrm.tile([128, 512], f32, tag="linv", name="linv")
                    lbc = [nrm.tile([128, 512], f32, tag=f"lbc{p}",
                                    name=f"lbc{p}") for p in range(2)]

                    def scores_exp(p, ki):
                        s_ps = scp.tile([128, 1024], f32, tag="s", name="s")
                        nc.tensor.matmul(
                            s_ps[:, 0:512],
                            lhsT=kr_sb[p][0:64, ki * 128:(ki + 1) * 128],
                            rhs=qr_sb[p][0:64, qs],
                            tile_position=(0, 0), start=True, stop=True)
                        nc.tensor.matmul(
                            s_ps[:, 512:1024],
                            lhsT=kr_sb[p][64:128, ki * 128:(ki + 1) * 128],
                            rhs=qr_sb[p][64:128, qs],
                            tile_position=(64, 0), start=True, stop=True)
                        if p == 0:
                            e_t = e0_sb[:, ki * 1024:(ki + 1) * 1024]
                        else:
                            e_t = esp.tile([128, 1024], bf16, tag="e1", name="e1")
                        nc.scalar.activation(e_t, s_ps[:, :], Exp)
                        return e_t

                    def emit_norm():
                        nc.vector.reciprocal(linv[:, :], den_ps[:, :])
                        lscr = ldr.tile([4, 512], f32, tag="lscr", name="lscr")
                        nc.sync.dma_start(lscr[:, :], linv[0:128:32, :])
                        for g, (p, half) in enumerate(
                                ((0, 0), (0, 1), (1, 0), (1, 1))):
                            nc.sync.dma_start(
                                lbc[p][half * 64:(half + 1) * 64, :],
                                lscr[g:g + 1, :].partition_broadcast(64))
                        for p in range(2):
                            nc.vector.tensor_tensor(
                                ctxn_sb[p][:, qs], ctx_ps[p][:, :],
                                lbc[p][:, :], mult)

                    for p in range(2):
                        if p == 1:
                            den_ps = dnp.tile([128, 512], f32, tag="den",
                                              name="den")
                            # unwritten partitions must stay finite for the
                            # reciprocal (only rows 0,32,64,96 are consumed)
                            nc.vector.memset(den_ps[:, :], 1.0)
                        fifo = collections.deque(F[(qh, p)])
                        popn = POPN[(qh, p)]
                        e_cur = scores_exp(p, 0)
                        for ki in range(ST):
                            e_this = e_cur
                            if ki < ST - 1:
                                e_cur = scores_exp(p, ki + 1)
                            for h2 in range(2):
                                vo = ki * E + (2 * p + h2) * 64
                                nc.tensor.matmul(
                                    ctx_ps[p][h2 * 64:(h2 + 1) * 64, :],
                                    lhsT=v_sb[:, vo:vo + 64],
                                    rhs=e_this[:, h2 * 512:(h2 + 1) * 512],
                                    tile_position=(0, h2 * 64),
                                    start=(ki == 0), stop=(ki == ST - 1),
                                    skip_group_check=True)
                            if p == 1:
                                # all four dens 4-way col-concurrent, using the
                                # stored pair-0 e tiles plus this fresh one
                                for g, (pp_, half) in enumerate(
                                        ((0, 0), (0, 1), (1, 0), (1, 1))):
                                    src = (e0_sb[:, ki * 1024:(ki + 1) * 1024]
                                           if pp_ == 0 else e_this)
                                    nc.tensor.matmul(
                                        den_ps[g * 32:g * 32 + 1, :],
                                        lhsT=ones_sb[:, :],
                                        rhs=src[:, half * 512:(half + 1) * 512],
                                        tile_position=(0, g * 32),
                                        start=(ki == 0), stop=(ki == ST - 1),
                                        skip_group_check=True)
                            for _ in range(popn):
                                if fifo:
                                    fifo.popleft()()
                        while fifo:
                            fifo.popleft()()
                    emit_norm()

                # epilogue: last qh's out-projection, ping-pong on scores slots
                for c in range(8):
                    run_all(outproj_chunk(scp, "s", 12 + c // 2, c % 2))

    _split_multiwait_instructions(nc, mybir, bass_rust.SyncInfo)
    return nc


def kernel(x_q, x_kv, wq, bq, wk, bk, wv, bv, wo, bo):
    from concourse import bass_utils

    x_q = np.asarray(x_q, dtype=np.float32)
    x_kv = np.asarray(x_kv, dtype=np.float32)
    wq = np.asarray(wq, dtype=np.float32); bq = np.asarray(bq, dtype=np.float32)
    wk = np.asarray(wk, dtype=np.float32); bk = np.asarray(bk, dtype=np.float32)
    wv = np.asarray(wv, dtype=np.float32); bv = np.asarray(bv, dtype=np.float32)
    wo = np.asarray(wo, dtype=np.float32); bo = np.asarray(bo, dtype=np.float32)

    with_bias = bool(bq.any() or bk.any() or bv.any())
    in_maps = _host_prep(x_q, x_kv, wq, bq, wk, bk, wv, bv, wo, with_bias)

    key = f"prog_{with_bias}"
    if key not in _PROGRAM_CACHE:
        _PROGRAM_CACHE[key] = build_program(with_bias)
    nc = _PROGRAM_CACHE[key]

    res = bass_utils.run_bass_kernel_spmd(
        nc, in_maps, core_ids=list(range(N_CORES)),
        trace=os.environ.get("KERNEL_TRACE", "") == "1")
    _PROGRAM_CACHE["last_result"] = res

    outp = np.zeros((B, S, D), dtype=np.float32)
    for c in range(N_CORES):
        outp[c // QUADS] += res.results[c]["out"]
    if bo.any():
        outp += bo[None, None, :]
    return outp
